# revision 2
# baseline (speedup 1.0000x reference)
"""Bass/Trainium2 kernel for nn_Exact_58454504899045 (GraftNet-style GNN).

Data-parallel over batch: 8 cores x 2 examples. One uniform SPMD program:
all data-dependent structure is normalized by padding facts into 128-fact
tiles aligned to 128-entity windows with a fixed tiles-per-window count.

Layouts:
  - entity tensors col-major (d on partitions, entity on free), EP=2048
  - fact tensors: col-major (101, FP) for matmul lhsT chunks; gathered/
    row-major tiles (128 facts, 128 cols) for DMA gather + merge matmuls
  - scatter by tail = per-tile merge matmul (lhsT = fact tile values,
    rhs = 0/1 merge block), accumulated in PSUM windows
  - gather by head = dma_gather of 512B staging rows from DRAM
    (staging row e = [hproj(e) | pagerank(e) | pr_ratio(e) | pad])
  - attention computed per 128-fact tile: sim matmul -> exp(ACT, accum sum)
    -> fused mul+reduce (DVE) giving wrapped (128, T) stats directly
  - e2f_softmax needs a scatter by head: separate head-sorted fact order
    with its own (cheap) attention pass + merge
"""
import os
import sys

sys.path.insert(0, "/opt/trn_rl_repo")

import numpy as np
import ml_dtypes

import concourse.bass as bass
import concourse.mybir as mybir
from concourse.tile import TileContext

FP32 = mybir.dt.float32
BF16 = mybir.dt.bfloat16
I16 = mybir.dt.int16
ALU = mybir.AluOpType
ACTF = mybir.ActivationFunctionType

NUM_ENTITY = 500000
NUM_RELATION = 6000
NUM_WORD = 200000
D = 100
L = 3
PAGERANK_LAMBDA = 0.8
FACT_SCALE = 3.0
VERY_NEG = -1e11
VERY_SMALL = 1e-10
E, F, Q = 2000, 8000, 20
EP = 2048
WINE = 128              # entity window (alignment of merge blocks)
NWIN = EP // WINE       # 16
TP = 128                # facts per tile
DIV = float(np.sqrt(D))

# ---------------------------------------------------------------- host side


def required_tpw(key_idx):
    """Max tiles (128 facts) needed by any 128-entity window."""
    cnt = np.bincount(key_idx // WINE, minlength=NWIN)
    return int(np.max((cnt + TP - 1) // TP))


def build_order(key_idx, tpw):
    """Facts sorted by key, packed into tpw tiles per 128-entity window.
    Returns slots (NWIN*tpw*128,) int32 orig fact index (-1 = pad)."""
    T = NWIN * tpw
    slots = np.full(T * TP, -1, np.int64)
    order = np.argsort(key_idx, kind="stable")
    k_s = key_idx[order]
    for w in range(NWIN):
        sel = order[(k_s // WINE) == w]
        base = w * tpw * TP
        assert len(sel) <= tpw * TP, "tiles-per-window overflow"
        slots[base:base + len(sel)] = sel
    return slots


def merge_matrix(slots, key_idx, tpw):
    """M (128, T*128) f32: per tile t, block [:,128t:128t+128] one-hot of
    local entity (key - window_base) for real slots."""
    T = NWIN * tpw
    M = np.zeros((TP, T * TP), np.float32)
    pos = np.arange(T * TP)
    valid = slots >= 0
    p = pos % TP
    t = pos // TP
    c = key_idx[np.maximum(slots, 0)] - (t // tpw) * WINE
    M[p[valid], (t * TP + c)[valid]] = 1.0
    return M


def wrap_idx(idx):
    """(N,) -> (128, N//16) int16, idx i at partition i%16 col i//16,
    replicated across the 8 gpsimd cores."""
    n = len(idx)
    assert n % 16 == 0
    a = np.ascontiguousarray(idx.astype(np.int16).reshape(n // 16, 16).T)
    return np.tile(a, (8, 1))


def _wimg_entries():
    """Static weight-image layout: list of (name, rows, cols)."""
    ent = []
    for i in range(L):
        ent += [(f"headT{i}", 101, D), (f"selfT{i}", 101, D),
                (f"tailT{i}", 100, D), (f"gAT{i}", 100, D),
                (f"gCT{i}", 100, D), (f"gBb{i}", 101, D),
                (f"eAT{i}", 100, D), (f"eCT{i}", 100, D),
                (f"eBb{i}", 101, D), (f"q2eT{i}", 101, D)]
    for g in range(4):
        ent += [(f"wihT{g}", 101, D), (f"whhT{g}", 100, D)]
    ent += [("scoreT", 101, 1), ("ones100", 1, D), ("eye16", 16, 16),
            ("zeros512", 1, 512), ("oh16", 1, 256),
            ("rlwNT", 100, D), ("rlb", 100, 1), ("e100", 128, 1)]
    return ent


def wimg_layout():
    off, lay = 0, {}
    for name, rows, cols in _wimg_entries():
        lay[name] = (off, rows, cols)
        off += cols
    total = max(off, 512)
    return lay, total


def build_wimg(w):
    lay, total = wimg_layout()
    img = np.zeros((128, total), np.float32)

    def put(name, arr):
        off, rows, cols = lay[name]
        assert arr.shape == (rows, cols), (name, arr.shape)
        img[:rows, off:off + cols] = arr

    for i in range(L):
        # self_b folded here: the gathered staging rows supply self_b to e2f
        put(f"headT{i}", np.vstack([w["head_w"][i].T,
                                    (w["head_b"][i] + w["self_b"][i])[None]]))
        put(f"selfT{i}", np.vstack([w["self_w"][i].T, w["self_b"][i][None]]))
        put(f"tailT{i}", FACT_SCALE * w["tail_w"][i].T)
        gA, gB, gC = (w["e2e_w"][i][:, :D], w["e2e_w"][i][:, D:2 * D],
                      w["e2e_w"][i][:, 2 * D:])
        eA, eB, eC = (w["e2q_w"][i][:, :D], w["e2q_w"][i][:, D:2 * D],
                      w["e2q_w"][i][:, 2 * D:])
        put(f"gAT{i}", gA.T)
        put(f"gCT{i}", gC.T)
        put(f"gBb{i}", np.vstack([gB.T, w["e2e_b"][i][None]]))
        put(f"eAT{i}", eA.T)
        put(f"eCT{i}", eC.T)
        put(f"eBb{i}", np.vstack([eB.T, w["e2q_b"][i][None]]))
        put(f"q2eT{i}", np.vstack([w["q2e_w"][i].T, w["q2e_b"][i][None]]))
    bias = w["lstm_bih"] + w["lstm_bhh"]
    for g in range(4):
        sl = slice(g * D, (g + 1) * D)
        put(f"wihT{g}", np.vstack([w["lstm_wih"][sl].T, bias[sl][None]]))
        put(f"whhT{g}", w["lstm_whh"][sl].T)
    put("scoreT", np.vstack([w["score_w"].T, w["score_b"][None]]))
    put("ones100", np.ones((1, D), np.float32))
    put("eye16", np.eye(16, dtype=np.float32))
    oh = np.zeros((1, 256), np.float32)
    oh[0, np.arange(16) * 16 + np.arange(16)] = 1.0
    put("oh16", oh)
    put("rlwNT", w["rel_lin_w"].astype(np.float32))
    put("rlb", w["rel_lin_b"][:, None].astype(np.float32))
    e100 = np.zeros((128, 1), np.float32)
    e100[100, 0] = 1.0
    put("e100", e100)
    return img


def build_rel16(w):
    """(6016, 128) bf16: row r = rel_table[r] | 1.0 | zeros."""
    img = np.zeros((6016, 128), ml_dtypes.bfloat16)
    img[:NUM_RELATION + 1, :D] = w["rel_table"].astype(ml_dtypes.bfloat16)
    img[:NUM_RELATION + 1, D] = 1.0
    return img


def build_bimg(w):
    """bf16 (128, L*D): selfR{i} = [(self_w@rel_lin_w).T; (self_w@rel_lin_b)]
    so A = raw_rel_gather @ selfR directly (fact_emb never materialized)."""
    img = np.zeros((128, L * D), ml_dtypes.bfloat16)
    for i in range(L):
        sr = (w["self_w"][i] @ w["rel_lin_w"]).T
        img[:D, i * D:(i + 1) * D] = sr.astype(ml_dtypes.bfloat16)
        img[D, i * D:(i + 1) * D] = (
            w["self_w"][i] @ w["rel_lin_b"]).astype(ml_dtypes.bfloat16)
    return img


def build_example(exi, w, tpw_t, tpw_h):
    """Host data for one example."""
    head, tail, rel = exi["kb_head"], exi["kb_tail"], exi["kb_fact_rel"]
    T_T, T_H = NWIN * tpw_t, NWIN * tpw_h
    slots_t = build_order(tail, tpw_t)
    slots_h = build_order(head, tpw_h)
    M = merge_matrix(slots_t, tail, tpw_t)
    Mh = merge_matrix(slots_h, head, tpw_h)
    rel_t = np.where(slots_t >= 0, rel[np.maximum(slots_t, 0)], NUM_RELATION)
    rel_h = np.where(slots_h >= 0, rel[np.maximum(slots_h, 0)], NUM_RELATION)
    head_t = np.where(slots_t >= 0, head[np.maximum(slots_t, 0)], EP - 1)

    le = np.zeros((102, EP), np.float32)
    le[:D, :E] = w["entity_table"][exi["local_entity"]].T
    le[D] = 1.0
    le[D + 1, :E] = exi["q2e_adj_mat"][:, 0]

    qe = np.zeros((101, Q), np.float32)
    qe[:D] = w["word_table"][exi["query_text"]].T
    qe[D] = 1.0

    return dict(M=M, Mh=Mh, idx_head=wrap_idx(head_t),
                idx_relt=wrap_idx(rel_t), idx_relh=wrap_idx(rel_h),
                le=le, qemb=qe,
                e_mask=(exi["local_entity"] != NUM_ENTITY).astype(np.float32),
                q_mask=(exi["query_text"] != NUM_WORD).astype(np.float32))


def build_core_inputs(ex0, ex1, wimg, T_T, T_H):
    m = np.concatenate([np.concatenate([e["M"], e["Mh"]], axis=1)
                        for e in (ex0, ex1)], axis=1)
    idx = np.concatenate([np.concatenate(
        [e["idx_head"], e["idx_relt"], e["idx_relh"]], axis=1)
        for e in (ex0, ex1)], axis=1)
    le = np.concatenate([ex0["le"], ex1["le"]], axis=1)
    qe = np.zeros((101, 2 * Q), np.float32)
    qe[:, 0::2] = ex0["qemb"]
    qe[:, 1::2] = ex1["qemb"]
    return {"wimg": wimg, "mimg": np.ascontiguousarray(m),
            "idx16": np.ascontiguousarray(idx),
            "leimg": np.ascontiguousarray(le),
            "qemb2": np.ascontiguousarray(qe)}


# ---------------------------------------------------------------- device side


def build_nc(T_T, T_H, rel16_shape):
    FPT, FPH = T_T * TP, T_H * TP
    lay, wtot = wimg_layout()
    nc = bass.Bass("TRN2")

    wimg_d = nc.dram_tensor("wimg", [128, wtot], FP32, kind="ExternalInput")
    mimg_d = nc.dram_tensor("mimg", [128, 2 * (FPT + FPH)], FP32,
                            kind="ExternalInput")
    idx_d = nc.dram_tensor("idx16", [128, 2 * (2 * FPT + FPH) // 16], I16,
                           kind="ExternalInput")
    le_d = nc.dram_tensor("leimg", [102, 2 * EP], FP32, kind="ExternalInput")
    qe_d = nc.dram_tensor("qemb2", [101, 2 * Q], FP32, kind="ExternalInput")
    rel16_d = nc.dram_tensor("rel16", list(rel16_shape), BF16,
                             kind="ExternalInput")
    bimg_d = nc.dram_tensor("bimg", [128, L * D], BF16, kind="ExternalInput")
    out_d = nc.dram_tensor("out", [2, EP], FP32, kind="ExternalOutput")

    # dma_gather is an extended GPSIMD instruction: load its ucode library
    # in the pre-Tile preamble so it precedes every gather.
    from concourse import library_config
    nc.gpsimd.load_library(library_config.mlp)

    with TileContext(nc) as tc:
        with (tc.tile_pool(name="const", bufs=1) as cpool,
              tc.tile_pool(name="big", bufs=1) as bpool,
              tc.tile_pool(name="work", bufs=1) as wpool,
              tc.tile_pool(name="small", bufs=2) as spool,
              tc.tile_pool(name="psum", bufs=3, space="PSUM") as pp,
              tc.tile_pool(name="psum4", bufs=4, space="PSUM") as pp4,
              tc.tile_pool(name="dram", bufs=1, space="DRAM") as dpool):

            # ---- constants
            W = cpool.tile([128, wtot], FP32, tag="wimg")
            nc.sync.dma_start(out=W[:], in_=wimg_d[:])

            def wsl(name, rows=None, cols=None):
                off, r, c = lay[name]
                return W[0:(rows or r), off:off + (cols or c)]

            zoff = lay["zeros512"][0]
            zero_l = W[0:1, zoff:zoff + 101]
            zero_r = W[0:1, zoff:zoff + 512]

            bimg = cpool.tile([128, L * D], BF16, tag="bimg")
            nc.sync.dma_start(out=bimg[:], in_=bimg_d[:])
            midx = cpool.tile([128, 2 * (2 * FPT + FPH) // 16], I16,
                              tag="midx")
            nc.sync.dma_start(out=midx[:], in_=idx_d[:])
            qe_sb = cpool.tile([101, 2 * Q], FP32, tag="qesb")
            nc.sync.dma_start(out=qe_sb[:], in_=qe_d[:])

            # ---- LSTM over both examples (interleaved t-major): hs (100,40)
            hs = cpool.tile([100, 2 * Q], FP32, tag="hs")
            ct = spool.tile([100, 2], FP32, tag="lstm_c")
            h0 = spool.tile([100, 2], FP32, tag="lstm_h0")
            nc.gpsimd.memset(ct[:], 0.0)
            nc.gpsimd.memset(h0[:], 0.0)
            xp = [cpool.tile([100, 2 * Q], FP32, tag=f"xp{g}", name=f"xp{g}")
                  for g in range(4)]
            for g in range(4):
                ps = pp.tile([100, 2 * Q], FP32, tag="ps1")
                nc.tensor.matmul(out=ps[:], lhsT=wsl(f"wihT{g}"),
                                 rhs=qe_sb[:], start=True, stop=True)
                nc.scalar.activation(out=xp[g][:], in_=ps[:], func=ACTF.Copy)
            gate = [spool.tile([100, 2], FP32, tag=f"gate{g}", name=f"gate{g}")
                    for g in range(4)]
            tmp1 = spool.tile([100, 2], FP32, tag="ltmp1")
            tmp2 = spool.tile([100, 2], FP32, tag="ltmp2")
            for t in range(Q):
                hin = h0[:] if t == 0 else hs[:, 2 * (t - 1):2 * t]
                for g in range(4):
                    ps = pp.tile([100, 2], FP32, tag="ps1")
                    nc.tensor.matmul(out=ps[:], lhsT=wsl(f"whhT{g}"),
                                     rhs=hin, start=True, stop=True)
                    nc.vector.scalar_tensor_tensor(
                        out=gate[g][:], in0=ps[:], scalar=0.0,
                        in1=xp[g][:, 2 * t:2 * t + 2],
                        op0=ALU.add, op1=ALU.add)
                for g, fn in ((0, ACTF.Sigmoid), (1, ACTF.Sigmoid),
                              (2, ACTF.Tanh), (3, ACTF.Sigmoid)):
                    nc.scalar.activation(out=gate[g][:], in_=gate[g][:],
                                         func=fn)
                nc.vector.tensor_tensor(out=tmp1[:], in0=gate[1][:],
                                        in1=ct[:], op=ALU.mult)
                nc.vector.tensor_tensor(out=tmp2[:], in0=gate[0][:],
                                        in1=gate[2][:], op=ALU.mult)
                nc.vector.tensor_tensor(out=ct[:], in0=tmp1[:], in1=tmp2[:],
                                        op=ALU.add)
                nc.scalar.activation(out=tmp1[:], in_=ct[:], func=ACTF.Tanh)
                nc.vector.tensor_tensor(out=hs[:, 2 * t:2 * t + 2],
                                        in0=gate[3][:], in1=tmp1[:],
                                        op=ALU.mult)
            qhs = cpool.tile([100, 2 * Q], FP32, tag="qhs")
            nc.vector.tensor_scalar_mul(qhs[:], hs[:], 1.0 / DIV)
            # qrel = [rel_lin_w^T @ qhs ; rel_lin_b @ qhs] as bf16 (101, 40):
            # sim_tile = (bf16 rel gather chunk).T @ qrel
            qrel = cpool.tile([101, 2 * Q], BF16, tag="qrel")
            psr = pp.tile([96, 2 * Q], FP32, tag="ps1")
            nc.tensor.matmul(out=psr[:], lhsT=wsl("rlwNT", cols=96),
                             rhs=qhs[:], start=True, stop=True)
            nc.scalar.activation(out=qrel[0:96, :], in_=psr[:],
                                 func=ACTF.Copy)
            roff = lay["rlwNT"][0]
            psr2 = pp.tile([5, 2 * Q], FP32, tag="ps1")
            nc.tensor.matmul(out=psr2[:], lhsT=W[0:100, roff + 96:roff + 101],
                             rhs=qhs[:], start=True, stop=True)
            nc.scalar.activation(out=qrel[96:101, :], in_=psr2[:],
                                 func=ACTF.Copy)

            # ---- shared big buffers (reused across examples)
            mbuf = bpool.tile([128, max(FPT, FPH)], FP32, tag="mbuf")
            f16 = bpool.tile([128, max(FPT, FPH)], BF16, tag="f16")
            G = bpool.tile([128, FPT], FP32, tag="G")
            stag = bpool.tile([128, 16 * TP], FP32, tag="stag")
            stag_d = dpool.tile([EP, TP], FP32, tag="stagd")
            LeA = cpool.tile([101, EP], FP32, tag="leA")
            LeB = cpool.tile([101, EP], FP32, tag="leB")
            # (1, EP) vectors packed into two 8-partition tiles (SBUF is
            # column-addressed; standalone 1-partition tiles waste 127/128)
            vecA = spool.tile([97, EP], FP32, tag="vecA", bufs=1)
            vecB = spool.tile([97, EP], FP32, tag="vecB", bufs=1)
            vecC = spool.tile([1, EP], FP32, tag="vecC", bufs=1)
            # 32-aligned bases; pager/esr at base 0 of different tiles so
            # tensor_tensor(pager, esr) has equal input bases; prr needs
            # base 0 too (wrap-matmul lhsT must match oh16 rhs base 0)
            pager = vecA[0:1, :]
            tprv = vecA[32:33, :]
            esr = vecB[0:1, :]
            esm = vecB[32:33, :]
            outsb = vecB[64:65, :]
            prr = vecC[0:1, :]
            f2e = wpool.tile([100, EP], FP32, tag="f2e")
            scat = wpool.tile([101, EP], FP32, tag="scat")
            Wt_t = spool.tile([128, T_T], FP32, tag="wtt", bufs=1)
            wpr = spool.tile([128, T_T], FP32, tag="wpr", bufs=1)
            qnode = spool.tile([101, 1], FP32, tag="qn", bufs=1)
            q2ec = spool.tile([101, 1], FP32, tag="q2ec", bufs=1)
            nc.gpsimd.memset(stag[:], 0.0)
            ooff = lay["ones100"][0]
            nc.sync.dma_start(out=LeB[100:101, :],
                              in_=le_d[100:101, 0:EP])
            nc.sync.dma_start(out=q2ec[100:101, 0:1],
                              in_=wimg_d[0:1, ooff:ooff + 1])
            nc.sync.dma_start(out=qnode[100:101, 0:1],
                              in_=wimg_d[0:1, ooff:ooff + 1])

            idx_base = [e * (2 * FPT + FPH) // 16 for e in (0, 1)]
            m_base = [e * (FPT + FPH) for e in (0, 1)]

            def attention(e, order, wt):
                T = T_T if order == "t" else T_H
                FPx = T * TP
                ib = idx_base[e] + (FPT // 16 if order == "t" else
                                    2 * FPT // 16)
                nc.gpsimd.dma_gather(
                    out_ap=f16[:, 0:FPx].rearrange("p (o f) -> p o f", o=1),
                    in_ap=rel16_d[:],
                    idxs_ap=midx[:, ib:ib + FPx // 16],
                    num_idxs=FPx, num_idxs_reg=FPx, elem_size=128,
                    transpose=True, single_packet=False)
                ssum = spool.tile([128, T], FP32, tag="ssum")
                wfn = spool.tile([128, T], FP32, tag="wfn")
                qsl = qrel[:].rearrange("p (t e) -> p t e", e=2)[:, :, e:e + 1]
                for t in range(T):
                    pss = pp.tile([128, Q], FP32, tag="ps1")
                    nc.tensor.matmul(out=pss[:],
                                     lhsT=f16[0:101, t * TP:(t + 1) * TP],
                                     rhs=qsl, start=True, stop=True)
                    es = spool.tile([128, Q], FP32, tag="es")
                    nc.scalar.activation(out=es[:], in_=pss[:], func=ACTF.Exp,
                                         accum_out=ssum[:, t:t + 1])
                    prod = spool.tile([128, Q], FP32, tag="prod")
                    nc.vector.scalar_tensor_tensor(
                        out=prod[:], in0=es[:], scalar=1.0, in1=pss[:],
                        op0=ALU.mult, op1=ALU.mult,
                        accum_out=wfn[:, t:t + 1])
                nc.vector.reciprocal(out=ssum[:], in_=ssum[:])
                nc.vector.tensor_tensor(out=wt[:, 0:T], in0=wfn[:],
                                        in1=ssum[:], op=ALU.mult)
                nc.scalar.activation(out=wt[:, 0:T], in_=wt[:, 0:T],
                                     func=ACTF.Exp)

            for e in (0, 1):
                # ---- per-example setup
                nc.sync.dma_start(out=LeA[0:101, :],
                                  in_=le_d[0:101, e * EP:(e + 1) * EP])
                nc.sync.dma_start(out=pager,
                                  in_=le_d[101:102, e * EP:(e + 1) * EP])
                nc.vector.tensor_copy(out=qnode[0:100, :],
                                      in_=hs[:, 2 * (Q - 1) + e:
                                             2 * (Q - 1) + e + 1])

                # head-order attention + e2f_softmax
                wth = spool.tile([128, T_H], FP32, tag="wth", bufs=1)
                attention(e, "h", wth)
                nc.sync.dma_start(
                    out=mbuf[:, 0:FPH],
                    in_=mimg_d[:, m_base[e] + FPT:m_base[e] + FPT + FPH])
                esP = [pp4.tile([1, 512], FP32, tag="scat", name="esP")
                       for _ in range(4)]
                for b in range(4):
                    nc.tensor.matmul(out=esP[b][:], lhsT=zero_l[:, 0:1],
                                     rhs=zero_r[:], start=True, stop=True,
                                     skip_group_check=True)
                tpw_h = T_H // NWIN
                for t in range(T_H):
                    w = t // tpw_h
                    nc.tensor.matmul(
                        out=esP[w // 4][0:1,
                                        WINE * (w % 4):WINE * (w % 4 + 1)],
                        lhsT=wth[:, t:t + 1],
                        rhs=mbuf[:, t * TP:(t + 1) * TP],
                        start=False, stop=True, skip_group_check=True)
                for b in range(4):
                    nc.vector.tensor_scalar_max(
                        esm[:, 512 * b:512 * (b + 1)], esP[b][:], VERY_SMALL)
                nc.vector.reciprocal(out=esr, in_=esm)

                # tail-order attention (materializes fact) + load M
                attention(e, "t", Wt_t)
                nc.sync.dma_start(out=mbuf[:, 0:FPT],
                                  in_=mimg_d[:, m_base[e]:m_base[e] + FPT])

                # ---- layers
                for i in range(L):
                    Le = LeA if i % 2 == 0 else LeB
                    Lenxt = LeB if i % 2 == 0 else LeA
                    psq = pp.tile([100, 1], FP32, tag="ps1")
                    nc.tensor.matmul(out=psq[:], lhsT=wsl(f"q2eT{i}"),
                                     rhs=qnode[:], start=True, stop=True)
                    nc.vector.tensor_copy(out=q2ec[0:100, :], in_=psq[:])
                    nc.vector.tensor_tensor(out=prr, in0=pager,
                                            in1=esr, op=ALU.mult)
                    for vec, col in ((pager, D), (prr, D + 1)):
                        psw = pp.tile([128, 16], FP32, tag="ps1")
                        for c in range(16):
                            nc.tensor.matmul(
                                out=psw[:], lhsT=vec[:, c * TP:(c + 1) * TP],
                                rhs=wsl("oh16")[0:1, 16 * c:16 * (c + 1)],
                                start=(c == 0), stop=(c == 15),
                                skip_group_check=True)
                        nc.vector.tensor_copy(
                            out=stag[:].rearrange(
                                "p (c j) -> p c j", j=TP)[:, :, col:col + 1],
                            in_=psw[:].rearrange("p (a b) -> p a b", b=1))
                    for c in range(16):
                        psh = pp.tile([128, D], FP32, tag="ps1")
                        nc.tensor.matmul(out=psh[:],
                                         lhsT=Le[:, c * TP:(c + 1) * TP],
                                         rhs=wsl(f"headT{i}"),
                                         start=True, stop=True)
                        nc.vector.tensor_copy(
                            out=stag[:, c * TP:c * TP + D], in_=psh[:])
                    nc.sync.dma_start(
                        out=stag_d[:].rearrange("(c p) j -> p c j", p=128),
                        in_=stag[:].rearrange("p (c j) -> p c j", j=TP))
                    nc.gpsimd.dma_gather(
                        out_ap=G[:].rearrange("p (t j) -> p t j", j=TP),
                        in_ap=stag_d[:],
                        idxs_ap=midx[:, idx_base[e]:idx_base[e] + FPT // 16],
                        num_idxs=FPT, num_idxs_reg=FPT, elem_size=128,
                        single_packet=False)
                    for t in range(T_T):
                        psa = pp.tile([128, D], FP32, tag="ps1")
                        nc.tensor.matmul(out=psa[:],
                                         lhsT=f16[0:101, t * TP:(t + 1) * TP],
                                         rhs=bimg[0:101, i * D:(i + 1) * D],
                                         start=True, stop=True)
                        gsl = G[:, t * TP:t * TP + D]
                        nc.vector.scalar_tensor_tensor(
                            out=gsl, in0=psa[:], scalar=0.0, in1=gsl,
                            op0=ALU.add, op1=ALU.add)
                    nc.vector.tensor_tensor(
                        out=wpr[:].rearrange("p (t j) -> p t j", j=1),
                        in0=Wt_t[:].rearrange("p (t j) -> p t j", j=1),
                        in1=G[:].rearrange(
                            "p (t j) -> p t j", j=TP)[:, :, D + 1:D + 2],
                        op=ALU.mult)
                    g3 = G[:].rearrange("p (t j) -> p t j", j=TP)
                    nc.vector.scalar_tensor_tensor(
                        out=g3[:, :, 0:D], in0=g3[:, :, 0:D], scalar=0.0,
                        in1=wpr[:].rearrange("p (t j) -> p t j", j=1)
                        .to_broadcast([128, T_T, D]),
                        op0=ALU.max, op1=ALU.mult)
                    scP = [pp4.tile([101, 512], FP32, tag="scat", name="scP")
                           for _ in range(4)]
                    for b in range(4):
                        nc.tensor.matmul(out=scP[b][:], lhsT=zero_l[:],
                                         rhs=zero_r[:], start=True, stop=True,
                                         skip_group_check=True)
                    tpw_t = T_T // NWIN
                    for t in range(T_T):
                        w = t // tpw_t
                        nc.tensor.matmul(
                            out=scP[w // 4][0:101,
                                            WINE * (w % 4):WINE * (w % 4 + 1)],
                            lhsT=G[:, t * TP:t * TP + 101],
                            rhs=mbuf[:, t * TP:(t + 1) * TP],
                            start=False, stop=True, skip_group_check=True)
                    for b in range(4):
                        nc.vector.tensor_copy(
                            out=scat[:, 512 * b:512 * (b + 1)],
                            in_=scP[b][:])
                    for c in range(4):
                        sl = slice(512 * c, 512 * (c + 1))
                        psf = pp.tile([100, 512], FP32, tag="ps1")
                        nc.tensor.matmul(out=psf[:], lhsT=wsl(f"selfT{i}"),
                                         rhs=Le[:, sl], start=True,
                                         stop=False, skip_group_check=True)
                        nc.tensor.matmul(out=psf[:], lhsT=wsl(f"tailT{i}"),
                                         rhs=scat[0:100, sl], start=False,
                                         stop=True, skip_group_check=True)
                        nc.scalar.activation(out=f2e[:, sl], in_=psf[:],
                                             func=ACTF.Relu)
                    nc.vector.tensor_scalar_mul(tprv, pager,
                                                1.0 - PAGERANK_LAMBDA)
                    for c in range(4):
                        sl = slice(512 * c, 512 * (c + 1))
                        psp = pp.tile([1, 512], FP32, tag="ps1")
                        nc.tensor.matmul(out=psp[:],
                                         lhsT=wsl("e100", rows=101),
                                         rhs=scat[0:101, sl],
                                         start=True, stop=True)
                        nc.vector.scalar_tensor_tensor(
                            out=pager[:, sl], in0=psp[:],
                            scalar=PAGERANK_LAMBDA, in1=tprv[:, sl],
                            op0=ALU.mult, op1=ALU.add)
                    s13 = spool.tile([100, 8], FP32, tag="s13")
                    scr = spool.tile([100, 512], FP32, tag="qscr", bufs=1)
                    for c in range(4):
                        sl = slice(512 * c, 512 * (c + 1))
                        psb = pp.tile([100, 512], FP32, tag="ps1")
                        nc.tensor.matmul(out=psb[:], lhsT=wsl("ones100"),
                                         rhs=pager[:, sl], start=True,
                                         stop=True)
                        nc.vector.scalar_tensor_tensor(
                            out=scr[:], in0=Le[0:100, sl], scalar=1.0,
                            in1=psb[:], op0=ALU.mult, op1=ALU.mult,
                            accum_out=s13[:, c:c + 1])
                        nc.vector.scalar_tensor_tensor(
                            out=scr[:], in0=f2e[:, sl], scalar=1.0,
                            in1=psb[:], op0=ALU.mult, op1=ALU.mult,
                            accum_out=s13[:, 4 + c:5 + c])
                    s1 = spool.tile([100, 2], FP32, tag="s1")
                    nc.vector.tensor_reduce(
                        out=s1[:],
                        in_=s13[:].rearrange("p (a b) -> p a b", b=4),
                        axis=mybir.AxisListType.X, op=ALU.add)
                    spr = spool.tile([1, 1], FP32, tag="spr")
                    nc.vector.tensor_reduce(out=spr[:], in_=pager,
                                            axis=mybir.AxisListType.X,
                                            op=ALU.add)
                    psq1 = pp.tile([100, 1], FP32, tag="ps1")
                    nc.tensor.matmul(out=psq1[:], lhsT=wsl(f"eAT{i}"),
                                     rhs=s1[:, 0:1], start=True, stop=False,
                                     skip_group_check=True)
                    nc.tensor.matmul(out=psq1[:], lhsT=wsl(f"eCT{i}"),
                                     rhs=s1[:, 1:2], start=False, stop=True,
                                     skip_group_check=True)
                    psq2 = pp.tile([100, 1], FP32, tag="ps1")
                    nc.tensor.matmul(out=psq2[:], lhsT=wsl(f"eBb{i}"),
                                     rhs=q2ec[:], start=True, stop=True)
                    psq3 = pp.tile([100, 1], FP32, tag="ps1")
                    nc.tensor.matmul(out=psq3[:], lhsT=wsl("ones100"),
                                     rhs=spr[:], start=True, stop=True)
                    sprb = spool.tile([100, 1], FP32, tag="sprb")
                    nc.vector.tensor_copy(out=sprb[:], in_=psq3[:])
                    tq = spool.tile([100, 1], FP32, tag="tq")
                    nc.vector.tensor_tensor(out=tq[:], in0=psq2[:],
                                            in1=sprb[:], op=ALU.mult)
                    nc.vector.tensor_tensor(out=qnode[0:100, :],
                                            in0=psq1[:], in1=tq[:],
                                            op=ALU.add)
                    psv = pp.tile([100, 1], FP32, tag="ps1")
                    nc.tensor.matmul(out=psv[:], lhsT=wsl(f"gBb{i}"),
                                     rhs=q2ec[:], start=True, stop=True)
                    biasc = spool.tile([100, 1], FP32, tag="biasc")
                    nc.vector.tensor_copy(out=biasc[:], in_=psv[:])
                    for c in range(4):
                        sl = slice(512 * c, 512 * (c + 1))
                        pse = pp.tile([100, 512], FP32, tag="ps1")
                        nc.tensor.matmul(out=pse[:], lhsT=wsl(f"gAT{i}"),
                                         rhs=Le[0:100, sl], start=True,
                                         stop=False, skip_group_check=True)
                        nc.tensor.matmul(out=pse[:], lhsT=wsl(f"gCT{i}"),
                                         rhs=f2e[:, sl], start=False,
                                         stop=True, skip_group_check=True)
                        nc.scalar.activation(out=Lenxt[0:100, sl],
                                             in_=pse[:], func=ACTF.Relu,
                                             bias=biasc[:])

                # ---- score
                Lefin = LeA if L % 2 == 0 else LeB
                for c in range(4):
                    sl = slice(512 * c, 512 * (c + 1))
                    pso = pp.tile([1, 512], FP32, tag="ps1")
                    nc.tensor.matmul(out=pso[:], lhsT=wsl("scoreT"),
                                     rhs=Lefin[:, sl], start=True,
                                     stop=True)
                    nc.vector.tensor_copy(out=outsb[:, sl], in_=pso[:])
                nc.sync.dma_start(out=out_d[e:e + 1, :], in_=outsb)

    # Raw Bass skips Bacc's codegen_inst_isa_subclasses; without it the
    # extended-inst InstISA subclasses (library reload) have empty bytes
    # and walrus fails with "ISA wrong length".
    from concourse.library_overlay import lower_extended_insts
    lower_extended_insts(nc)
    return nc



# ---------------------------------------------------------------- runner


def prepare(inputs):
    names = ["local_entity", "kb_fact_rel", "kb_head", "kb_tail",
             "query_text", "q2e_adj_mat"]
    w = {k: np.asarray(v, np.float32) for k, v in inputs.items()
         if k not in names}
    per = {n: np.asarray(inputs[n]) for n in names}
    B = per["local_entity"].shape[0]
    tpw_t = max(required_tpw(per["kb_tail"][b]) for b in range(B))
    tpw_h = max(required_tpw(per["kb_head"][b]) for b in range(B))
    T_T, T_H = NWIN * tpw_t, NWIN * tpw_h
    wimg = build_wimg(w)
    rel16 = build_rel16(w)
    bimg = build_bimg(w)
    exd = [build_example({n: per[n][b] for n in names}, w, tpw_t, tpw_h)
           for b in range(B)]
    in_maps = []
    for c in range(B // 2):
        im = build_core_inputs(exd[2 * c], exd[2 * c + 1], wimg, T_T, T_H)
        im["rel16"] = rel16
        im["bimg"] = bimg
        in_maps.append(im)
    e_mask = np.stack([e["e_mask"] for e in exd])
    return in_maps, T_T, T_H, rel16.shape, e_mask


_IN_NAMES = ["wimg", "mimg", "idx16", "leimg", "qemb2", "rel16", "bimg"]
_CACHE_DIR = os.environ.get(
    "BKERN_CACHE", os.path.expanduser("~/.cache/nnx58454504899045"))
_MEM = {}


def _export_path(key):
    return os.path.join(_CACHE_DIR, f"exp_{key}.bin")


def _build_exported(T_T, T_H, rel16_shape, sample_maps):
    import jax
    from jax.sharding import Mesh, PartitionSpec
    from jax.experimental.shard_map import shard_map
    from concourse.bass2jax import (
        _bass_exec_p, partition_id_tensor, install_neuronx_cc_hook,
        _fast_dispatch_active)
    install_neuronx_cc_hook()
    nc = build_nc(T_T, T_H, rel16_shape)
    pn = nc.partition_id_tensor.name if nc.partition_id_tensor else None
    out_name, out_shape = "out", (2, EP)
    out_avals = [jax.core.ShapedArray(out_shape, np.float32)]
    in_names_full = list(_IN_NAMES) + [out_name] + ([pn] if pn else [])

    def _body(*args):
        operands = list(args)
        if pn:
            operands.append(partition_id_tensor())
        outs = _bass_exec_p.bind(
            *operands, out_avals=tuple(out_avals),
            in_names=tuple(in_names_full), out_names=(out_name,),
            lowering_input_output_aliases=(), sim_require_finite=True,
            sim_require_nnan=True, nc=nc)
        return tuple(outs)

    devices = jax.devices()[:8]
    mesh = Mesh(np.asarray(devices), ("core",))
    nin = len(_IN_NAMES)
    f = jax.jit(
        shard_map(_body, mesh=mesh,
                  in_specs=(PartitionSpec("core"),) * (nin + 1),
                  out_specs=(PartitionSpec("core"),), check_rep=False),
        donate_argnums=(nin,), keep_unused=True)
    gl = [np.concatenate([m[n] for m in sample_maps], axis=0)
          for n in _IN_NAMES]
    zout = np.zeros((8 * 2, EP), np.float32)
    with _fast_dispatch_active(True):
        exported = jax.export.export(
            f, disabled_checks=[
                jax.export.DisabledSafetyCheck.custom_call("bass_exec")]
        )(*gl, zout)
    return exported


def _get_exported(T_T, T_H, rel16_shape, sample_maps):
    key = f"{T_T}_{T_H}_{rel16_shape[0]}_v5"
    if key in _MEM:
        return _MEM[key]
    import jax
    path = _export_path(key)
    exported = None
    if os.path.exists(path):
        try:
            from concourse.bass2jax import install_neuronx_cc_hook
            install_neuronx_cc_hook()
            exported = jax.export.deserialize(open(path, "rb").read())
        except Exception:
            exported = None
    if exported is None:
        exported = _build_exported(T_T, T_H, rel16_shape, sample_maps)
        try:
            os.makedirs(_CACHE_DIR, exist_ok=True)
            tmp = path + ".tmp"
            open(tmp, "wb").write(exported.serialize())
            os.replace(tmp, path)
        except Exception:
            pass
    _MEM[key] = exported
    return exported


def run_device(in_maps, T_T, T_H, rel16_shape):
    import jax
    from jax.sharding import Mesh, PartitionSpec, NamedSharding
    exported = _get_exported(T_T, T_H, rel16_shape, in_maps)
    mesh = Mesh(np.asarray(jax.devices()[:8]), ("core",))
    sh = NamedSharding(mesh, PartitionSpec("core"))
    gl = [jax.device_put(np.concatenate([m[n] for m in in_maps], axis=0), sh)
          for n in _IN_NAMES]
    zout = jax.device_put(np.zeros((8 * 2, EP), np.float32), sh)
    callf = jax.jit(exported.call)
    res = callf(*gl, zout)
    return np.asarray(res[0])


def kernel(**inputs):
    in_maps, T_T, T_H, rel16_shape, e_mask = prepare(inputs)
    out = run_device(in_maps, T_T, T_H, rel16_shape)   # (16, EP)
    score = out[:, :E]
    return (score + (1.0 - e_mask) * VERY_NEG).astype(np.float32)


# revision 3
# speedup vs baseline: 1.0128x; 1.0128x over previous
"""Bass/Trainium2 kernel for nn_Exact_58454504899045 (GraftNet-style GNN).

Data-parallel over batch: 8 cores x 2 examples. One uniform SPMD program:
all data-dependent structure is normalized by padding facts into 128-fact
tiles aligned to 128-entity windows with a fixed tiles-per-window count.

Layouts:
  - entity tensors col-major (d on partitions, entity on free), EP=2048
  - fact tensors: col-major (101, FP) for matmul lhsT chunks; gathered/
    row-major tiles (128 facts, 128 cols) for DMA gather + merge matmuls
  - scatter by tail = per-tile merge matmul (lhsT = fact tile values,
    rhs = 0/1 merge block), accumulated in PSUM windows
  - gather by head = dma_gather of 512B staging rows from DRAM
    (staging row e = [hproj(e) | pagerank(e) | pr_ratio(e) | pad])
  - attention computed per 128-fact tile: sim matmul -> exp(ACT, accum sum)
    -> fused mul+reduce (DVE) giving wrapped (128, T) stats directly
  - e2f_softmax needs a scatter by head: separate head-sorted fact order
    with its own (cheap) attention pass + merge
"""
import os
import sys

sys.path.insert(0, "/opt/trn_rl_repo")

import numpy as np
import ml_dtypes

import concourse.bass as bass
import concourse.mybir as mybir
from concourse.tile import TileContext

FP32 = mybir.dt.float32
BF16 = mybir.dt.bfloat16
I16 = mybir.dt.int16
ALU = mybir.AluOpType
ACTF = mybir.ActivationFunctionType

NUM_ENTITY = 500000
NUM_RELATION = 6000
NUM_WORD = 200000
D = 100
L = 3
PAGERANK_LAMBDA = 0.8
FACT_SCALE = 3.0
VERY_NEG = -1e11
VERY_SMALL = 1e-10
E, F, Q = 2000, 8000, 20
EP = 2048
WINE = 128              # entity window (alignment of merge blocks)
NWIN = EP // WINE       # 16
TP = 128                # facts per tile
DIV = float(np.sqrt(D))

# ---------------------------------------------------------------- host side


def required_tpw(key_idx):
    """Max tiles (128 facts) needed by any 128-entity window."""
    cnt = np.bincount(key_idx // WINE, minlength=NWIN)
    return int(np.max((cnt + TP - 1) // TP))


def build_order(key_idx, tpw):
    """Facts sorted by key, packed into tpw tiles per 128-entity window.
    Returns slots (NWIN*tpw*128,) int32 orig fact index (-1 = pad)."""
    T = NWIN * tpw
    slots = np.full(T * TP, -1, np.int64)
    order = np.argsort(key_idx, kind="stable")
    k_s = key_idx[order]
    for w in range(NWIN):
        sel = order[(k_s // WINE) == w]
        base = w * tpw * TP
        assert len(sel) <= tpw * TP, "tiles-per-window overflow"
        slots[base:base + len(sel)] = sel
    return slots


def merge_matrix(slots, key_idx, tpw):
    """M (128, T*128) f32: per tile t, block [:,128t:128t+128] one-hot of
    local entity (key - window_base) for real slots."""
    T = NWIN * tpw
    M = np.zeros((TP, T * TP), np.uint8)
    pos = np.arange(T * TP)
    valid = slots >= 0
    p = pos % TP
    t = pos // TP
    c = key_idx[np.maximum(slots, 0)] - (t // tpw) * WINE
    M[p[valid], (t * TP + c)[valid]] = 1
    return M


def wrap_idx(idx):
    """(N,) -> (128, N//16) int16, idx i at partition i%16 col i//16,
    replicated across the 8 gpsimd cores."""
    n = len(idx)
    assert n % 16 == 0
    a = np.ascontiguousarray(idx.astype(np.int16).reshape(n // 16, 16).T)
    return np.tile(a, (8, 1))


def _wimg_entries():
    """Static weight-image layout: list of (name, rows, cols)."""
    ent = []
    for i in range(L):
        ent += [(f"headT{i}", 101, D), (f"selfT{i}", 101, D),
                (f"tailT{i}", 100, D), (f"gAT{i}", 100, D),
                (f"gCT{i}", 100, D), (f"gBb{i}", 101, D),
                (f"eAT{i}", 100, D), (f"eCT{i}", 100, D),
                (f"eBb{i}", 101, D), (f"q2eT{i}", 101, D)]
    for g in range(4):
        ent += [(f"wihT{g}", 101, D), (f"whhT{g}", 100, D)]
    ent += [("scoreT", 101, 1), ("ones100", 1, D), ("eye16", 16, 16),
            ("zeros512", 1, 512), ("oh16", 1, 256),
            ("rlwNT", 100, D), ("rlb", 100, 1), ("e100", 128, 1)]
    return ent


def wimg_layout():
    off, lay = 0, {}
    for name, rows, cols in _wimg_entries():
        lay[name] = (off, rows, cols)
        off += cols
    total = max(off, 512)
    return lay, total


def build_wimg(w):
    lay, total = wimg_layout()
    img = np.zeros((128, total), np.float32)

    def put(name, arr):
        off, rows, cols = lay[name]
        assert arr.shape == (rows, cols), (name, arr.shape)
        img[:rows, off:off + cols] = arr

    for i in range(L):
        # self_b folded here: the gathered staging rows supply self_b to e2f
        put(f"headT{i}", np.vstack([w["head_w"][i].T,
                                    (w["head_b"][i] + w["self_b"][i])[None]]))
        put(f"selfT{i}", np.vstack([w["self_w"][i].T, w["self_b"][i][None]]))
        put(f"tailT{i}", FACT_SCALE * w["tail_w"][i].T)
        gA, gB, gC = (w["e2e_w"][i][:, :D], w["e2e_w"][i][:, D:2 * D],
                      w["e2e_w"][i][:, 2 * D:])
        eA, eB, eC = (w["e2q_w"][i][:, :D], w["e2q_w"][i][:, D:2 * D],
                      w["e2q_w"][i][:, 2 * D:])
        put(f"gAT{i}", gA.T)
        put(f"gCT{i}", gC.T)
        put(f"gBb{i}", np.vstack([gB.T, w["e2e_b"][i][None]]))
        put(f"eAT{i}", eA.T)
        put(f"eCT{i}", eC.T)
        put(f"eBb{i}", np.vstack([eB.T, w["e2q_b"][i][None]]))
        put(f"q2eT{i}", np.vstack([w["q2e_w"][i].T, w["q2e_b"][i][None]]))
    bias = w["lstm_bih"] + w["lstm_bhh"]
    for g in range(4):
        sl = slice(g * D, (g + 1) * D)
        put(f"wihT{g}", np.vstack([w["lstm_wih"][sl].T, bias[sl][None]]))
        put(f"whhT{g}", w["lstm_whh"][sl].T)
    put("scoreT", np.vstack([w["score_w"].T, w["score_b"][None]]))
    put("ones100", np.ones((1, D), np.float32))
    put("eye16", np.eye(16, dtype=np.float32))
    oh = np.zeros((1, 256), np.float32)
    oh[0, np.arange(16) * 16 + np.arange(16)] = 1.0
    put("oh16", oh)
    put("rlwNT", w["rel_lin_w"].astype(np.float32))
    put("rlb", w["rel_lin_b"][:, None].astype(np.float32))
    e100 = np.zeros((128, 1), np.float32)
    e100[100, 0] = 1.0
    put("e100", e100)
    return img


def build_rel16(w):
    """(6016, 128) bf16: row r = rel_table[r] | 1.0 | zeros."""
    img = np.zeros((6016, 128), ml_dtypes.bfloat16)
    img[:NUM_RELATION + 1, :D] = w["rel_table"].astype(ml_dtypes.bfloat16)
    img[:NUM_RELATION + 1, D] = 1.0
    return img


def build_bimg(w):
    """bf16 (128, L*D): selfR{i} = [(self_w@rel_lin_w).T; (self_w@rel_lin_b)]
    so A = raw_rel_gather @ selfR directly (fact_emb never materialized)."""
    img = np.zeros((128, L * D), ml_dtypes.bfloat16)
    for i in range(L):
        sr = (w["self_w"][i] @ w["rel_lin_w"]).T
        img[:D, i * D:(i + 1) * D] = sr.astype(ml_dtypes.bfloat16)
        img[D, i * D:(i + 1) * D] = (
            w["self_w"][i] @ w["rel_lin_b"]).astype(ml_dtypes.bfloat16)
    return img


def build_example(exi, w, tpw_t, tpw_h):
    """Host data for one example."""
    head, tail, rel = exi["kb_head"], exi["kb_tail"], exi["kb_fact_rel"]
    T_T, T_H = NWIN * tpw_t, NWIN * tpw_h
    slots_t = build_order(tail, tpw_t)
    slots_h = build_order(head, tpw_h)
    M = merge_matrix(slots_t, tail, tpw_t)
    Mh = merge_matrix(slots_h, head, tpw_h)
    rel_t = np.where(slots_t >= 0, rel[np.maximum(slots_t, 0)], NUM_RELATION)
    rel_h = np.where(slots_h >= 0, rel[np.maximum(slots_h, 0)], NUM_RELATION)
    head_t = np.where(slots_t >= 0, head[np.maximum(slots_t, 0)], EP - 1)

    le = np.zeros((102, EP), np.float32)
    le[:D, :E] = w["entity_table"][exi["local_entity"]].T
    le[D] = 1.0
    le[D + 1, :E] = exi["q2e_adj_mat"][:, 0]

    qe = np.zeros((101, Q), np.float32)
    qe[:D] = w["word_table"][exi["query_text"]].T
    qe[D] = 1.0

    return dict(M=M, Mh=Mh, idx_head=wrap_idx(head_t),
                idx_relt=wrap_idx(rel_t), idx_relh=wrap_idx(rel_h),
                le=le, qemb=qe,
                e_mask=(exi["local_entity"] != NUM_ENTITY).astype(np.float32),
                q_mask=(exi["query_text"] != NUM_WORD).astype(np.float32))


def build_core_inputs(ex0, ex1, wimg, T_T, T_H):
    m = np.concatenate([np.concatenate([e["M"], e["Mh"]], axis=1)
                        for e in (ex0, ex1)], axis=1)
    idx = np.concatenate([np.concatenate(
        [e["idx_head"], e["idx_relt"], e["idx_relh"]], axis=1)
        for e in (ex0, ex1)], axis=1)
    le = np.concatenate([ex0["le"], ex1["le"]], axis=1)
    qe = np.zeros((101, 2 * Q), np.float32)
    qe[:, 0::2] = ex0["qemb"]
    qe[:, 1::2] = ex1["qemb"]
    return {"wimg": wimg, "mimg": np.ascontiguousarray(m),
            "idx16": np.ascontiguousarray(idx),
            "leimg": np.ascontiguousarray(le),
            "qemb2": np.ascontiguousarray(qe)}


# ---------------------------------------------------------------- device side


def build_nc(T_T, T_H, rel16_shape):
    FPT, FPH = T_T * TP, T_H * TP
    lay, wtot = wimg_layout()
    nc = bass.Bass("TRN2")

    wimg_d = nc.dram_tensor("wimg", [128, wtot], FP32, kind="ExternalInput")
    mimg_d = nc.dram_tensor("mimg", [128, 2 * (FPT + FPH)], mybir.dt.uint8,
                            kind="ExternalInput")
    idx_d = nc.dram_tensor("idx16", [128, 2 * (2 * FPT + FPH) // 16], I16,
                           kind="ExternalInput")
    le_d = nc.dram_tensor("leimg", [102, 2 * EP], FP32, kind="ExternalInput")
    qe_d = nc.dram_tensor("qemb2", [101, 2 * Q], FP32, kind="ExternalInput")
    rel16_d = nc.dram_tensor("rel16", list(rel16_shape), BF16,
                             kind="ExternalInput")
    bimg_d = nc.dram_tensor("bimg", [128, L * D], BF16, kind="ExternalInput")
    out_d = nc.dram_tensor("out", [2, EP], FP32, kind="ExternalOutput")

    # dma_gather is an extended GPSIMD instruction: load its ucode library
    # in the pre-Tile preamble so it precedes every gather.
    from concourse import library_config
    nc.gpsimd.load_library(library_config.mlp)

    with TileContext(nc) as tc:
        with (tc.tile_pool(name="const", bufs=1) as cpool,
              tc.tile_pool(name="big", bufs=1) as bpool,
              tc.tile_pool(name="work", bufs=1) as wpool,
              tc.tile_pool(name="small", bufs=2) as spool,
              tc.tile_pool(name="psum", bufs=3, space="PSUM") as pp,
              tc.tile_pool(name="psum4", bufs=4, space="PSUM") as pp4,
              tc.tile_pool(name="dram", bufs=1, space="DRAM") as dpool):

            # ---- constants
            W = cpool.tile([128, wtot], FP32, tag="wimg")
            nc.sync.dma_start(out=W[:], in_=wimg_d[:])

            def wsl(name, rows=None, cols=None):
                off, r, c = lay[name]
                return W[0:(rows or r), off:off + (cols or c)]

            zoff = lay["zeros512"][0]
            zero_l = W[0:1, zoff:zoff + 101]
            zero_r = W[0:1, zoff:zoff + 512]

            bimg = cpool.tile([128, L * D], BF16, tag="bimg")
            nc.sync.dma_start(out=bimg[:], in_=bimg_d[:])
            midx = cpool.tile([128, 2 * (2 * FPT + FPH) // 16], I16,
                              tag="midx")
            nc.sync.dma_start(out=midx[:], in_=idx_d[:])
            qe_sb = cpool.tile([101, 2 * Q], FP32, tag="qesb")
            nc.sync.dma_start(out=qe_sb[:], in_=qe_d[:])

            # ---- LSTM over both examples (interleaved t-major): hs (100,40)
            hs = cpool.tile([100, 2 * Q], FP32, tag="hs")
            ct = spool.tile([100, 2], FP32, tag="lstm_c")
            h0 = spool.tile([100, 2], FP32, tag="lstm_h0")
            nc.gpsimd.memset(ct[:], 0.0)
            nc.gpsimd.memset(h0[:], 0.0)
            xp = [cpool.tile([100, 2 * Q], FP32, tag=f"xp{g}", name=f"xp{g}")
                  for g in range(4)]
            for g in range(4):
                ps = pp.tile([100, 2 * Q], FP32, tag="ps1")
                nc.tensor.matmul(out=ps[:], lhsT=wsl(f"wihT{g}"),
                                 rhs=qe_sb[:], start=True, stop=True)
                nc.scalar.activation(out=xp[g][:], in_=ps[:], func=ACTF.Copy)
            gate = [spool.tile([100, 2], FP32, tag=f"gate{g}", name=f"gate{g}")
                    for g in range(4)]
            tmp1 = spool.tile([100, 2], FP32, tag="ltmp1")
            tmp2 = spool.tile([100, 2], FP32, tag="ltmp2")
            for t in range(Q):
                hin = h0[:] if t == 0 else hs[:, 2 * (t - 1):2 * t]
                for g in range(4):
                    ps = pp.tile([100, 2], FP32, tag="ps1")
                    nc.tensor.matmul(out=ps[:], lhsT=wsl(f"whhT{g}"),
                                     rhs=hin, start=True, stop=True)
                    nc.vector.scalar_tensor_tensor(
                        out=gate[g][:], in0=ps[:], scalar=0.0,
                        in1=xp[g][:, 2 * t:2 * t + 2],
                        op0=ALU.add, op1=ALU.add)
                for g, fn in ((0, ACTF.Sigmoid), (1, ACTF.Sigmoid),
                              (2, ACTF.Tanh), (3, ACTF.Sigmoid)):
                    nc.scalar.activation(out=gate[g][:], in_=gate[g][:],
                                         func=fn)
                nc.vector.tensor_tensor(out=tmp1[:], in0=gate[1][:],
                                        in1=ct[:], op=ALU.mult)
                nc.vector.tensor_tensor(out=tmp2[:], in0=gate[0][:],
                                        in1=gate[2][:], op=ALU.mult)
                nc.vector.tensor_tensor(out=ct[:], in0=tmp1[:], in1=tmp2[:],
                                        op=ALU.add)
                nc.scalar.activation(out=tmp1[:], in_=ct[:], func=ACTF.Tanh)
                nc.vector.tensor_tensor(out=hs[:, 2 * t:2 * t + 2],
                                        in0=gate[3][:], in1=tmp1[:],
                                        op=ALU.mult)
            qhs = cpool.tile([100, 2 * Q], FP32, tag="qhs")
            nc.vector.tensor_scalar_mul(qhs[:], hs[:], 1.0 / DIV)
            # qrel = [rel_lin_w^T @ qhs ; rel_lin_b @ qhs] as bf16 (101, 40):
            # sim_tile = (bf16 rel gather chunk).T @ qrel
            qrel = cpool.tile([101, 2 * Q], BF16, tag="qrel")
            psr = pp.tile([96, 2 * Q], FP32, tag="ps1")
            nc.tensor.matmul(out=psr[:], lhsT=wsl("rlwNT", cols=96),
                             rhs=qhs[:], start=True, stop=True)
            nc.scalar.activation(out=qrel[0:96, :], in_=psr[:],
                                 func=ACTF.Copy)
            roff = lay["rlwNT"][0]
            psr2 = pp.tile([5, 2 * Q], FP32, tag="ps1")
            nc.tensor.matmul(out=psr2[:], lhsT=W[0:100, roff + 96:roff + 101],
                             rhs=qhs[:], start=True, stop=True)
            nc.scalar.activation(out=qrel[96:101, :], in_=psr2[:],
                                 func=ACTF.Copy)

            # ---- shared big buffers (reused across examples)
            mbuf = bpool.tile([128, max(FPT, FPH)], FP32, tag="mbuf")
            f16 = bpool.tile([128, max(FPT, FPH)], BF16, tag="f16")
            G = bpool.tile([128, FPT], FP32, tag="G")
            stag = bpool.tile([128, 16 * TP], FP32, tag="stag")
            stag_d = dpool.tile([EP, TP], FP32, tag="stagd")
            LeA = cpool.tile([101, EP], FP32, tag="leA")
            LeB = cpool.tile([101, EP], FP32, tag="leB")
            # (1, EP) vectors packed into two 8-partition tiles (SBUF is
            # column-addressed; standalone 1-partition tiles waste 127/128)
            vecA = spool.tile([97, EP], FP32, tag="vecA", bufs=1)
            vecB = spool.tile([97, EP], FP32, tag="vecB", bufs=1)
            vecC = spool.tile([1, EP], FP32, tag="vecC", bufs=1)
            # 32-aligned bases; pager/esr at base 0 of different tiles so
            # tensor_tensor(pager, esr) has equal input bases; prr needs
            # base 0 too (wrap-matmul lhsT must match oh16 rhs base 0)
            pager = vecA[0:1, :]
            tprv = vecA[32:33, :]
            esr = vecB[0:1, :]
            esm = vecB[32:33, :]
            outsb = vecB[64:65, :]
            prr = vecC[0:1, :]
            f2e = wpool.tile([100, EP], FP32, tag="f2e")
            scat = wpool.tile([101, EP], FP32, tag="scat")
            Wt_t = spool.tile([128, T_T], FP32, tag="wtt", bufs=1)
            wpr = spool.tile([128, T_T], FP32, tag="wpr", bufs=1)
            qnode = spool.tile([101, 1], FP32, tag="qn", bufs=1)
            q2ec = spool.tile([101, 1], FP32, tag="q2ec", bufs=1)
            nc.gpsimd.memset(stag[:], 0.0)
            ooff = lay["ones100"][0]
            nc.sync.dma_start(out=LeB[100:101, :],
                              in_=le_d[100:101, 0:EP])
            nc.sync.dma_start(out=q2ec[100:101, 0:1],
                              in_=wimg_d[0:1, ooff:ooff + 1])
            nc.sync.dma_start(out=qnode[100:101, 0:1],
                              in_=wimg_d[0:1, ooff:ooff + 1])

            idx_base = [e * (2 * FPT + FPH) // 16 for e in (0, 1)]
            m_base = [e * (FPT + FPH) for e in (0, 1)]

            def attention(e, order, wt):
                T = T_T if order == "t" else T_H
                FPx = T * TP
                ib = idx_base[e] + (FPT // 16 if order == "t" else
                                    2 * FPT // 16)
                nc.gpsimd.dma_gather(
                    out_ap=f16[:, 0:FPx].rearrange("p (o f) -> p o f", o=1),
                    in_ap=rel16_d[:],
                    idxs_ap=midx[:, ib:ib + FPx // 16],
                    num_idxs=FPx, num_idxs_reg=FPx, elem_size=128,
                    transpose=True, single_packet=False)
                ssum = spool.tile([128, T], FP32, tag="ssum")
                wfn = spool.tile([128, T], FP32, tag="wfn")
                qsl = qrel[:].rearrange("p (t e) -> p t e", e=2)[:, :, e:e + 1]
                for t in range(T):
                    pss = pp.tile([128, Q], FP32, tag="ps1")
                    nc.tensor.matmul(out=pss[:],
                                     lhsT=f16[0:101, t * TP:(t + 1) * TP],
                                     rhs=qsl, start=True, stop=True)
                    es = spool.tile([128, Q], FP32, tag="es")
                    nc.scalar.activation(out=es[:], in_=pss[:], func=ACTF.Exp,
                                         accum_out=ssum[:, t:t + 1])
                    prod = spool.tile([128, Q], FP32, tag="prod")
                    nc.vector.scalar_tensor_tensor(
                        out=prod[:], in0=es[:], scalar=1.0, in1=pss[:],
                        op0=ALU.mult, op1=ALU.mult,
                        accum_out=wfn[:, t:t + 1])
                nc.vector.reciprocal(out=ssum[:], in_=ssum[:])
                nc.vector.tensor_tensor(out=wt[:, 0:T], in0=wfn[:],
                                        in1=ssum[:], op=ALU.mult)
                nc.scalar.activation(out=wt[:, 0:T], in_=wt[:, 0:T],
                                     func=ACTF.Exp)

            for e in (0, 1):
                # ---- per-example setup
                nc.sync.dma_start(out=LeA[0:101, :],
                                  in_=le_d[0:101, e * EP:(e + 1) * EP])
                nc.sync.dma_start(out=pager,
                                  in_=le_d[101:102, e * EP:(e + 1) * EP])
                nc.vector.tensor_copy(out=qnode[0:100, :],
                                      in_=hs[:, 2 * (Q - 1) + e:
                                             2 * (Q - 1) + e + 1])

                # head-order attention + e2f_softmax
                wth = spool.tile([128, T_H], FP32, tag="wth", bufs=1)
                attention(e, "h", wth)
                nc.gpsimd.dma_start(
                    out=mbuf[:, 0:FPH],
                    in_=mimg_d[:, m_base[e] + FPT:m_base[e] + FPT + FPH])
                esP = [pp4.tile([1, 512], FP32, tag="scat", name="esP")
                       for _ in range(4)]
                for b in range(4):
                    nc.tensor.matmul(out=esP[b][:], lhsT=zero_l[:, 0:1],
                                     rhs=zero_r[:], start=True, stop=True,
                                     skip_group_check=True)
                tpw_h = T_H // NWIN
                for t in range(T_H):
                    w = t // tpw_h
                    nc.tensor.matmul(
                        out=esP[w // 4][0:1,
                                        WINE * (w % 4):WINE * (w % 4 + 1)],
                        lhsT=wth[:, t:t + 1],
                        rhs=mbuf[:, t * TP:(t + 1) * TP],
                        start=False, stop=True, skip_group_check=True)
                for b in range(4):
                    nc.vector.tensor_scalar_max(
                        esm[:, 512 * b:512 * (b + 1)], esP[b][:], VERY_SMALL)
                nc.vector.reciprocal(out=esr, in_=esm)

                # tail-order attention (materializes fact) + load M
                attention(e, "t", Wt_t)
                nc.gpsimd.dma_start(out=mbuf[:, 0:FPT],
                                  in_=mimg_d[:, m_base[e]:m_base[e] + FPT])

                # ---- layers
                for i in range(L):
                    Le = LeA if i % 2 == 0 else LeB
                    Lenxt = LeB if i % 2 == 0 else LeA
                    psq = pp.tile([100, 1], FP32, tag="ps1")
                    nc.tensor.matmul(out=psq[:], lhsT=wsl(f"q2eT{i}"),
                                     rhs=qnode[:], start=True, stop=True)
                    nc.vector.tensor_copy(out=q2ec[0:100, :], in_=psq[:])
                    nc.vector.tensor_tensor(out=prr, in0=pager,
                                            in1=esr, op=ALU.mult)
                    for vec, col in ((pager, D), (prr, D + 1)):
                        psw = pp.tile([128, 16], FP32, tag="ps1")
                        for c in range(16):
                            nc.tensor.matmul(
                                out=psw[:], lhsT=vec[:, c * TP:(c + 1) * TP],
                                rhs=wsl("oh16")[0:1, 16 * c:16 * (c + 1)],
                                start=(c == 0), stop=(c == 15),
                                skip_group_check=True)
                        nc.vector.tensor_copy(
                            out=stag[:].rearrange(
                                "p (c j) -> p c j", j=TP)[:, :, col:col + 1],
                            in_=psw[:].rearrange("p (a b) -> p a b", b=1))
                    for c in range(16):
                        psh = pp.tile([128, D], FP32, tag="ps1")
                        nc.tensor.matmul(out=psh[:],
                                         lhsT=Le[:, c * TP:(c + 1) * TP],
                                         rhs=wsl(f"headT{i}"),
                                         start=True, stop=True)
                        nc.vector.tensor_copy(
                            out=stag[:, c * TP:c * TP + D], in_=psh[:])
                    nc.sync.dma_start(
                        out=stag_d[:].rearrange("(c p) j -> p c j", p=128),
                        in_=stag[:].rearrange("p (c j) -> p c j", j=TP))
                    nc.gpsimd.dma_gather(
                        out_ap=G[:].rearrange("p (t j) -> p t j", j=TP),
                        in_ap=stag_d[:],
                        idxs_ap=midx[:, idx_base[e]:idx_base[e] + FPT // 16],
                        num_idxs=FPT, num_idxs_reg=FPT, elem_size=128,
                        single_packet=False)
                    for t in range(T_T):
                        psa = pp.tile([128, D], FP32, tag="ps1")
                        nc.tensor.matmul(out=psa[:],
                                         lhsT=f16[0:101, t * TP:(t + 1) * TP],
                                         rhs=bimg[0:101, i * D:(i + 1) * D],
                                         start=True, stop=True)
                        gsl = G[:, t * TP:t * TP + D]
                        nc.vector.scalar_tensor_tensor(
                            out=gsl, in0=psa[:], scalar=0.0, in1=gsl,
                            op0=ALU.add, op1=ALU.add)
                    nc.vector.tensor_tensor(
                        out=wpr[:].rearrange("p (t j) -> p t j", j=1),
                        in0=Wt_t[:].rearrange("p (t j) -> p t j", j=1),
                        in1=G[:].rearrange(
                            "p (t j) -> p t j", j=TP)[:, :, D + 1:D + 2],
                        op=ALU.mult)
                    g3 = G[:].rearrange("p (t j) -> p t j", j=TP)
                    nc.vector.scalar_tensor_tensor(
                        out=g3[:, :, 0:D], in0=g3[:, :, 0:D], scalar=0.0,
                        in1=wpr[:].rearrange("p (t j) -> p t j", j=1)
                        .to_broadcast([128, T_T, D]),
                        op0=ALU.max, op1=ALU.mult)
                    scP = [pp4.tile([101, 512], FP32, tag="scat", name="scP")
                           for _ in range(4)]
                    for b in range(4):
                        nc.tensor.matmul(out=scP[b][:], lhsT=zero_l[:],
                                         rhs=zero_r[:], start=True, stop=True,
                                         skip_group_check=True)
                    tpw_t = T_T // NWIN
                    for t in range(T_T):
                        w = t // tpw_t
                        nc.tensor.matmul(
                            out=scP[w // 4][0:101,
                                            WINE * (w % 4):WINE * (w % 4 + 1)],
                            lhsT=G[:, t * TP:t * TP + 101],
                            rhs=mbuf[:, t * TP:(t + 1) * TP],
                            start=False, stop=True, skip_group_check=True)
                    for b in range(4):
                        nc.vector.tensor_copy(
                            out=scat[:, 512 * b:512 * (b + 1)],
                            in_=scP[b][:])
                    for c in range(4):
                        sl = slice(512 * c, 512 * (c + 1))
                        psf = pp.tile([100, 512], FP32, tag="ps1")
                        nc.tensor.matmul(out=psf[:], lhsT=wsl(f"selfT{i}"),
                                         rhs=Le[:, sl], start=True,
                                         stop=False, skip_group_check=True)
                        nc.tensor.matmul(out=psf[:], lhsT=wsl(f"tailT{i}"),
                                         rhs=scat[0:100, sl], start=False,
                                         stop=True, skip_group_check=True)
                        nc.scalar.activation(out=f2e[:, sl], in_=psf[:],
                                             func=ACTF.Relu)
                    nc.vector.tensor_scalar_mul(tprv, pager,
                                                1.0 - PAGERANK_LAMBDA)
                    for c in range(4):
                        sl = slice(512 * c, 512 * (c + 1))
                        psp = pp.tile([1, 512], FP32, tag="ps1")
                        nc.tensor.matmul(out=psp[:],
                                         lhsT=wsl("e100", rows=101),
                                         rhs=scat[0:101, sl],
                                         start=True, stop=True)
                        nc.vector.scalar_tensor_tensor(
                            out=pager[:, sl], in0=psp[:],
                            scalar=PAGERANK_LAMBDA, in1=tprv[:, sl],
                            op0=ALU.mult, op1=ALU.add)
                    s13 = spool.tile([100, 8], FP32, tag="s13")
                    scr = spool.tile([100, 512], FP32, tag="qscr", bufs=1)
                    for c in range(4):
                        sl = slice(512 * c, 512 * (c + 1))
                        psb = pp.tile([100, 512], FP32, tag="ps1")
                        nc.tensor.matmul(out=psb[:], lhsT=wsl("ones100"),
                                         rhs=pager[:, sl], start=True,
                                         stop=True)
                        nc.vector.scalar_tensor_tensor(
                            out=scr[:], in0=Le[0:100, sl], scalar=1.0,
                            in1=psb[:], op0=ALU.mult, op1=ALU.mult,
                            accum_out=s13[:, c:c + 1])
                        nc.vector.scalar_tensor_tensor(
                            out=scr[:], in0=f2e[:, sl], scalar=1.0,
                            in1=psb[:], op0=ALU.mult, op1=ALU.mult,
                            accum_out=s13[:, 4 + c:5 + c])
                    s1 = spool.tile([100, 2], FP32, tag="s1")
                    nc.vector.tensor_reduce(
                        out=s1[:],
                        in_=s13[:].rearrange("p (a b) -> p a b", b=4),
                        axis=mybir.AxisListType.X, op=ALU.add)
                    spr = spool.tile([1, 1], FP32, tag="spr")
                    nc.vector.tensor_reduce(out=spr[:], in_=pager,
                                            axis=mybir.AxisListType.X,
                                            op=ALU.add)
                    psq1 = pp.tile([100, 1], FP32, tag="ps1")
                    nc.tensor.matmul(out=psq1[:], lhsT=wsl(f"eAT{i}"),
                                     rhs=s1[:, 0:1], start=True, stop=False,
                                     skip_group_check=True)
                    nc.tensor.matmul(out=psq1[:], lhsT=wsl(f"eCT{i}"),
                                     rhs=s1[:, 1:2], start=False, stop=True,
                                     skip_group_check=True)
                    psq2 = pp.tile([100, 1], FP32, tag="ps1")
                    nc.tensor.matmul(out=psq2[:], lhsT=wsl(f"eBb{i}"),
                                     rhs=q2ec[:], start=True, stop=True)
                    psq3 = pp.tile([100, 1], FP32, tag="ps1")
                    nc.tensor.matmul(out=psq3[:], lhsT=wsl("ones100"),
                                     rhs=spr[:], start=True, stop=True)
                    sprb = spool.tile([100, 1], FP32, tag="sprb")
                    nc.vector.tensor_copy(out=sprb[:], in_=psq3[:])
                    tq = spool.tile([100, 1], FP32, tag="tq")
                    nc.vector.tensor_tensor(out=tq[:], in0=psq2[:],
                                            in1=sprb[:], op=ALU.mult)
                    nc.vector.tensor_tensor(out=qnode[0:100, :],
                                            in0=psq1[:], in1=tq[:],
                                            op=ALU.add)
                    psv = pp.tile([100, 1], FP32, tag="ps1")
                    nc.tensor.matmul(out=psv[:], lhsT=wsl(f"gBb{i}"),
                                     rhs=q2ec[:], start=True, stop=True)
                    biasc = spool.tile([100, 1], FP32, tag="biasc")
                    nc.vector.tensor_copy(out=biasc[:], in_=psv[:])
                    for c in range(4):
                        sl = slice(512 * c, 512 * (c + 1))
                        pse = pp.tile([100, 512], FP32, tag="ps1")
                        nc.tensor.matmul(out=pse[:], lhsT=wsl(f"gAT{i}"),
                                         rhs=Le[0:100, sl], start=True,
                                         stop=False, skip_group_check=True)
                        nc.tensor.matmul(out=pse[:], lhsT=wsl(f"gCT{i}"),
                                         rhs=f2e[:, sl], start=False,
                                         stop=True, skip_group_check=True)
                        nc.scalar.activation(out=Lenxt[0:100, sl],
                                             in_=pse[:], func=ACTF.Relu,
                                             bias=biasc[:])

                # ---- score
                Lefin = LeA if L % 2 == 0 else LeB
                for c in range(4):
                    sl = slice(512 * c, 512 * (c + 1))
                    pso = pp.tile([1, 512], FP32, tag="ps1")
                    nc.tensor.matmul(out=pso[:], lhsT=wsl("scoreT"),
                                     rhs=Lefin[:, sl], start=True,
                                     stop=True)
                    nc.vector.tensor_copy(out=outsb[:, sl], in_=pso[:])
                nc.sync.dma_start(out=out_d[e:e + 1, :], in_=outsb)

    # Raw Bass skips Bacc's codegen_inst_isa_subclasses; without it the
    # extended-inst InstISA subclasses (library reload) have empty bytes
    # and walrus fails with "ISA wrong length".
    from concourse.library_overlay import lower_extended_insts
    lower_extended_insts(nc)
    return nc



# ---------------------------------------------------------------- runner


def prepare(inputs):
    names = ["local_entity", "kb_fact_rel", "kb_head", "kb_tail",
             "query_text", "q2e_adj_mat"]
    w = {k: np.asarray(v, np.float32) for k, v in inputs.items()
         if k not in names}
    per = {n: np.asarray(inputs[n]) for n in names}
    B = per["local_entity"].shape[0]
    tpw_t = max(required_tpw(per["kb_tail"][b]) for b in range(B))
    tpw_h = max(required_tpw(per["kb_head"][b]) for b in range(B))
    T_T, T_H = NWIN * tpw_t, NWIN * tpw_h
    wimg = build_wimg(w)
    rel16 = build_rel16(w)
    bimg = build_bimg(w)
    exd = [build_example({n: per[n][b] for n in names}, w, tpw_t, tpw_h)
           for b in range(B)]
    in_maps = []
    for c in range(B // 2):
        im = build_core_inputs(exd[2 * c], exd[2 * c + 1], wimg, T_T, T_H)
        im["rel16"] = rel16
        im["bimg"] = bimg
        in_maps.append(im)
    e_mask = np.stack([e["e_mask"] for e in exd])
    return in_maps, T_T, T_H, rel16.shape, e_mask


_IN_NAMES = ["wimg", "mimg", "idx16", "leimg", "qemb2", "rel16", "bimg"]
_CACHE_DIR = os.environ.get(
    "BKERN_CACHE", os.path.expanduser("~/.cache/nnx58454504899045"))
_MEM = {}


def _export_path(key):
    return os.path.join(_CACHE_DIR, f"exp_{key}.bin")


def _build_exported(T_T, T_H, rel16_shape, sample_maps):
    import jax
    from jax.sharding import Mesh, PartitionSpec
    from jax.experimental.shard_map import shard_map
    from concourse.bass2jax import (
        _bass_exec_p, partition_id_tensor, install_neuronx_cc_hook,
        _fast_dispatch_active)
    install_neuronx_cc_hook()
    nc = build_nc(T_T, T_H, rel16_shape)
    pn = nc.partition_id_tensor.name if nc.partition_id_tensor else None
    out_name, out_shape = "out", (2, EP)
    out_avals = [jax.core.ShapedArray(out_shape, np.float32)]
    in_names_full = list(_IN_NAMES) + [out_name] + ([pn] if pn else [])

    def _body(*args):
        operands = list(args)
        if pn:
            operands.append(partition_id_tensor())
        outs = _bass_exec_p.bind(
            *operands, out_avals=tuple(out_avals),
            in_names=tuple(in_names_full), out_names=(out_name,),
            lowering_input_output_aliases=(), sim_require_finite=True,
            sim_require_nnan=True, nc=nc)
        return tuple(outs)

    devices = jax.devices()[:8]
    mesh = Mesh(np.asarray(devices), ("core",))
    nin = len(_IN_NAMES)
    f = jax.jit(
        shard_map(_body, mesh=mesh,
                  in_specs=(PartitionSpec("core"),) * (nin + 1),
                  out_specs=(PartitionSpec("core"),), check_rep=False),
        donate_argnums=(nin,), keep_unused=True)
    gl = [np.concatenate([m[n] for m in sample_maps], axis=0)
          for n in _IN_NAMES]
    zout = np.zeros((8 * 2, EP), np.float32)
    with _fast_dispatch_active(True):
        exported = jax.export.export(
            f, disabled_checks=[
                jax.export.DisabledSafetyCheck.custom_call("bass_exec")]
        )(*gl, zout)
    return exported


def _get_exported(T_T, T_H, rel16_shape, sample_maps):
    key = f"{T_T}_{T_H}_{rel16_shape[0]}_v6"
    if key in _MEM:
        return _MEM[key]
    import jax
    path = _export_path(key)
    exported = None
    if os.path.exists(path):
        try:
            from concourse.bass2jax import install_neuronx_cc_hook
            install_neuronx_cc_hook()
            exported = jax.export.deserialize(open(path, "rb").read())
        except Exception:
            exported = None
    if exported is None:
        exported = _build_exported(T_T, T_H, rel16_shape, sample_maps)
        try:
            os.makedirs(_CACHE_DIR, exist_ok=True)
            tmp = path + ".tmp"
            open(tmp, "wb").write(exported.serialize())
            os.replace(tmp, path)
        except Exception:
            pass
    _MEM[key] = exported
    return exported


def run_device(in_maps, T_T, T_H, rel16_shape):
    import jax
    from jax.sharding import Mesh, PartitionSpec, NamedSharding
    exported = _get_exported(T_T, T_H, rel16_shape, in_maps)
    mesh = Mesh(np.asarray(jax.devices()[:8]), ("core",))
    sh = NamedSharding(mesh, PartitionSpec("core"))
    gl = [jax.device_put(np.concatenate([m[n] for m in in_maps], axis=0), sh)
          for n in _IN_NAMES]
    zout = jax.device_put(np.zeros((8 * 2, EP), np.float32), sh)
    callf = jax.jit(exported.call)
    res = callf(*gl, zout)
    return np.asarray(res[0])


def kernel(**inputs):
    in_maps, T_T, T_H, rel16_shape, e_mask = prepare(inputs)
    out = run_device(in_maps, T_T, T_H, rel16_shape)   # (16, EP)
    score = out[:, :E]
    return (score + (1.0 - e_mask) * VERY_NEG).astype(np.float32)


# revision 4
# speedup vs baseline: 1.0458x; 1.0326x over previous
"""Bass/Trainium2 kernel for nn_Exact_58454504899045 (GraftNet-style GNN).

Data-parallel over batch: 8 cores x 2 examples. One uniform SPMD program:
all data-dependent structure is normalized by padding facts into 128-fact
tiles aligned to 128-entity windows with a fixed tiles-per-window count.

Layouts:
  - entity tensors col-major (d on partitions, entity on free), EP=2048
  - fact tensors: col-major (101, FP) for matmul lhsT chunks; gathered/
    row-major tiles (128 facts, 128 cols) for DMA gather + merge matmuls
  - scatter by tail = per-tile merge matmul (lhsT = fact tile values,
    rhs = 0/1 merge block), accumulated in PSUM windows
  - gather by head = dma_gather of 512B staging rows from DRAM
    (staging row e = [hproj(e) | pagerank(e) | pr_ratio(e) | pad])
  - attention computed per 128-fact tile: sim matmul -> exp(ACT, accum sum)
    -> fused mul+reduce (DVE) giving wrapped (128, T) stats directly
  - e2f_softmax needs a scatter by head: separate head-sorted fact order
    with its own (cheap) attention pass + merge
"""
import os
import sys

sys.path.insert(0, "/opt/trn_rl_repo")

import numpy as np
import ml_dtypes

import concourse.bass as bass
import concourse.mybir as mybir
from concourse.tile import TileContext

FP32 = mybir.dt.float32
BF16 = mybir.dt.bfloat16
I16 = mybir.dt.int16
ALU = mybir.AluOpType
ACTF = mybir.ActivationFunctionType

NUM_ENTITY = 500000
NUM_RELATION = 6000
NUM_WORD = 200000
D = 100
L = 3
PAGERANK_LAMBDA = 0.8
FACT_SCALE = 3.0
VERY_NEG = -1e11
VERY_SMALL = 1e-10
E, F, Q = 2000, 8000, 20
EP = 2048
WINE = 128              # entity window (alignment of merge blocks)
NWIN = EP // WINE       # 16
TP = 128                # facts per tile
DIV = float(np.sqrt(D))

# ---------------------------------------------------------------- host side


def required_tpw(key_idx):
    """Max tiles (128 facts) needed by any 128-entity window."""
    cnt = np.bincount(key_idx // WINE, minlength=NWIN)
    return int(np.max((cnt + TP - 1) // TP))


def build_order(key_idx, tpw):
    """Facts sorted by key, packed into tpw tiles per 128-entity window.
    Returns slots (NWIN*tpw*128,) int32 orig fact index (-1 = pad)."""
    T = NWIN * tpw
    slots = np.full(T * TP, -1, np.int64)
    order = np.argsort(key_idx, kind="stable")
    k_s = key_idx[order]
    for w in range(NWIN):
        sel = order[(k_s // WINE) == w]
        base = w * tpw * TP
        assert len(sel) <= tpw * TP, "tiles-per-window overflow"
        slots[base:base + len(sel)] = sel
    return slots


def merge_matrix(slots, key_idx, tpw):
    """M (128, T*128) f32: per tile t, block [:,128t:128t+128] one-hot of
    local entity (key - window_base) for real slots."""
    T = NWIN * tpw
    M = np.zeros((TP, T * TP), np.uint8)
    pos = np.arange(T * TP)
    valid = slots >= 0
    p = pos % TP
    t = pos // TP
    c = key_idx[np.maximum(slots, 0)] - (t // tpw) * WINE
    M[p[valid], (t * TP + c)[valid]] = 1
    return M


def wrap_idx(idx):
    """(N,) -> (128, N//16) int16, idx i at partition i%16 col i//16,
    replicated across the 8 gpsimd cores."""
    n = len(idx)
    assert n % 16 == 0
    a = np.ascontiguousarray(idx.astype(np.int16).reshape(n // 16, 16).T)
    return np.tile(a, (8, 1))


def _wimg_entries():
    """Static weight-image layout: list of (name, rows, cols)."""
    ent = []
    for i in range(L):
        ent += [(f"headT{i}", 101, D), (f"selfT{i}", 101, D),
                (f"tailT{i}", 100, D), (f"gAT{i}", 100, D),
                (f"gCT{i}", 100, D), (f"gBb{i}", 101, D),
                (f"eAT{i}", 100, D), (f"eCT{i}", 100, D),
                (f"eBb{i}", 101, D), (f"q2eT{i}", 101, D)]
    for g in range(4):
        ent += [(f"wihT{g}", 101, D), (f"whhT{g}", 100, D)]
    ent += [("scoreT", 101, 1), ("ones100", 1, D), ("eye16", 16, 16),
            ("zeros512", 1, 512), ("oh16", 1, 256),
            ("rlwNT", 100, D), ("rlb", 100, 1), ("e100", 128, 1)]
    return ent


def wimg_layout():
    off, lay = 0, {}
    for name, rows, cols in _wimg_entries():
        lay[name] = (off, rows, cols)
        off += cols
    total = max(off, 512)
    return lay, total


def build_wimg(w):
    lay, total = wimg_layout()
    img = np.zeros((128, total), np.float32)

    def put(name, arr):
        off, rows, cols = lay[name]
        assert arr.shape == (rows, cols), (name, arr.shape)
        img[:rows, off:off + cols] = arr

    for i in range(L):
        # self_b folded here: the gathered staging rows supply self_b to e2f
        put(f"headT{i}", np.vstack([w["head_w"][i].T,
                                    (w["head_b"][i] + w["self_b"][i])[None]]))
        put(f"selfT{i}", np.vstack([w["self_w"][i].T, w["self_b"][i][None]]))
        put(f"tailT{i}", FACT_SCALE * w["tail_w"][i].T)
        gA, gB, gC = (w["e2e_w"][i][:, :D], w["e2e_w"][i][:, D:2 * D],
                      w["e2e_w"][i][:, 2 * D:])
        eA, eB, eC = (w["e2q_w"][i][:, :D], w["e2q_w"][i][:, D:2 * D],
                      w["e2q_w"][i][:, 2 * D:])
        put(f"gAT{i}", gA.T)
        put(f"gCT{i}", gC.T)
        put(f"gBb{i}", np.vstack([gB.T, w["e2e_b"][i][None]]))
        put(f"eAT{i}", eA.T)
        put(f"eCT{i}", eC.T)
        put(f"eBb{i}", np.vstack([eB.T, w["e2q_b"][i][None]]))
        put(f"q2eT{i}", np.vstack([w["q2e_w"][i].T, w["q2e_b"][i][None]]))
    bias = w["lstm_bih"] + w["lstm_bhh"]
    for g in range(4):
        sl = slice(g * D, (g + 1) * D)
        put(f"wihT{g}", np.vstack([w["lstm_wih"][sl].T, bias[sl][None]]))
        put(f"whhT{g}", w["lstm_whh"][sl].T)
    put("scoreT", np.vstack([w["score_w"].T, w["score_b"][None]]))
    put("ones100", np.ones((1, D), np.float32))
    put("eye16", np.eye(16, dtype=np.float32))
    oh = np.zeros((1, 256), np.float32)
    oh[0, np.arange(16) * 16 + np.arange(16)] = 1.0
    put("oh16", oh)
    put("rlwNT", w["rel_lin_w"].astype(np.float32))
    put("rlb", w["rel_lin_b"][:, None].astype(np.float32))
    e100 = np.zeros((128, 1), np.float32)
    e100[100, 0] = 1.0
    put("e100", e100)
    return img


def build_rel16(w):
    """(6016, 128) bf16: row r = rel_table[r] | 1.0 | zeros."""
    img = np.zeros((6016, 128), ml_dtypes.bfloat16)
    img[:NUM_RELATION + 1, :D] = w["rel_table"].astype(ml_dtypes.bfloat16)
    img[:NUM_RELATION + 1, D] = 1.0
    return img


def build_bimg(w):
    """bf16 (128, L*D): selfR{i} = [(self_w@rel_lin_w).T; (self_w@rel_lin_b)]
    so A = raw_rel_gather @ selfR directly (fact_emb never materialized)."""
    img = np.zeros((128, L * D), ml_dtypes.bfloat16)
    for i in range(L):
        sr = (w["self_w"][i] @ w["rel_lin_w"]).T
        img[:D, i * D:(i + 1) * D] = sr.astype(ml_dtypes.bfloat16)
        img[D, i * D:(i + 1) * D] = (
            w["self_w"][i] @ w["rel_lin_b"]).astype(ml_dtypes.bfloat16)
    return img


def build_example(exi, w, tpw_t, tpw_h):
    """Host data for one example."""
    head, tail, rel = exi["kb_head"], exi["kb_tail"], exi["kb_fact_rel"]
    T_T, T_H = NWIN * tpw_t, NWIN * tpw_h
    slots_t = build_order(tail, tpw_t)
    slots_h = build_order(head, tpw_h)
    M = merge_matrix(slots_t, tail, tpw_t)
    Mh = merge_matrix(slots_h, head, tpw_h)
    rel_t = np.where(slots_t >= 0, rel[np.maximum(slots_t, 0)], NUM_RELATION)
    rel_h = np.where(slots_h >= 0, rel[np.maximum(slots_h, 0)], NUM_RELATION)
    head_t = np.where(slots_t >= 0, head[np.maximum(slots_t, 0)], EP - 1)

    le = np.zeros((102, EP), np.float32)
    le[:D, :E] = w["entity_table"][exi["local_entity"]].T
    le[D] = 1.0
    le[D + 1, :E] = exi["q2e_adj_mat"][:, 0]

    qe = np.zeros((101, Q), np.float32)
    qe[:D] = w["word_table"][exi["query_text"]].T
    qe[D] = 1.0

    return dict(M=M, Mh=Mh, idx_head=wrap_idx(head_t),
                idx_relt=wrap_idx(rel_t), idx_relh=wrap_idx(rel_h),
                le=le, qemb=qe,
                e_mask=(exi["local_entity"] != NUM_ENTITY).astype(np.float32),
                q_mask=(exi["query_text"] != NUM_WORD).astype(np.float32))


def build_core_inputs(ex0, ex1, wimg, T_T, T_H):
    m = np.concatenate([np.concatenate([e["M"], e["Mh"]], axis=1)
                        for e in (ex0, ex1)], axis=1)
    idx = np.concatenate([np.concatenate(
        [e["idx_head"], e["idx_relt"], e["idx_relh"]], axis=1)
        for e in (ex0, ex1)], axis=1)
    le = np.concatenate([ex0["le"], ex1["le"]], axis=1)
    qe = np.zeros((101, 2 * Q), np.float32)
    qe[:, 0::2] = ex0["qemb"]
    qe[:, 1::2] = ex1["qemb"]
    return {"wimg": wimg, "mimg": np.ascontiguousarray(m),
            "idx16": np.ascontiguousarray(idx),
            "leimg": np.ascontiguousarray(le),
            "qemb2": np.ascontiguousarray(qe)}


# ---------------------------------------------------------------- device side


def build_nc(T_T, T_H, rel16_shape):
    FPT, FPH = T_T * TP, T_H * TP
    lay, wtot = wimg_layout()
    nc = bass.Bass("TRN2")

    wimg_d = nc.dram_tensor("wimg", [128, wtot], FP32, kind="ExternalInput")
    mimg_d = nc.dram_tensor("mimg", [128, 2 * (FPT + FPH)], mybir.dt.uint8,
                            kind="ExternalInput")
    idx_d = nc.dram_tensor("idx16", [128, 2 * (2 * FPT + FPH) // 16], I16,
                           kind="ExternalInput")
    le_d = nc.dram_tensor("leimg", [102, 2 * EP], FP32, kind="ExternalInput")
    qe_d = nc.dram_tensor("qemb2", [101, 2 * Q], FP32, kind="ExternalInput")
    rel16_d = nc.dram_tensor("rel16", list(rel16_shape), BF16,
                             kind="ExternalInput")
    bimg_d = nc.dram_tensor("bimg", [128, L * D], BF16, kind="ExternalInput")
    out_d = nc.dram_tensor("out", [2, EP], FP32, kind="ExternalOutput")

    # dma_gather is an extended GPSIMD instruction: load its ucode library
    # in the pre-Tile preamble so it precedes every gather.
    from concourse import library_config
    nc.gpsimd.load_library(library_config.mlp)

    with TileContext(nc) as tc:
        with (tc.tile_pool(name="const", bufs=1) as cpool,
              tc.tile_pool(name="big", bufs=1) as bpool,
              tc.tile_pool(name="work", bufs=1) as wpool,
              tc.tile_pool(name="small", bufs=2) as spool,
              tc.tile_pool(name="psum", bufs=3, space="PSUM") as pp,
              tc.tile_pool(name="psum4", bufs=4, space="PSUM") as pp4,
              tc.tile_pool(name="dram", bufs=1, space="DRAM") as dpool):

            # ---- constants
            W = cpool.tile([128, wtot], FP32, tag="wimg")
            nc.sync.dma_start(out=W[:], in_=wimg_d[:])

            def wsl(name, rows=None, cols=None):
                off, r, c = lay[name]
                return W[0:(rows or r), off:off + (cols or c)]

            zoff = lay["zeros512"][0]
            zero_l = W[0:1, zoff:zoff + 101]
            zero_r = W[0:1, zoff:zoff + 512]

            bimg = cpool.tile([128, L * D], BF16, tag="bimg")
            nc.sync.dma_start(out=bimg[:], in_=bimg_d[:])
            midx = cpool.tile([128, 2 * (2 * FPT + FPH) // 16], I16,
                              tag="midx")
            nc.sync.dma_start(out=midx[:], in_=idx_d[:])
            qe_sb = cpool.tile([101, 2 * Q], FP32, tag="qesb")
            nc.sync.dma_start(out=qe_sb[:], in_=qe_d[:])

            # ---- LSTM over both examples (interleaved t-major): hs (100,40)
            hs = cpool.tile([100, 2 * Q], FP32, tag="hs")
            ct = spool.tile([100, 2], FP32, tag="lstm_c")
            h0 = spool.tile([100, 2], FP32, tag="lstm_h0")
            nc.gpsimd.memset(ct[:], 0.0)
            nc.gpsimd.memset(h0[:], 0.0)
            xp = [cpool.tile([100, 2 * Q], FP32, tag=f"xp{g}", name=f"xp{g}")
                  for g in range(4)]
            for g in range(4):
                ps = pp.tile([100, 2 * Q], FP32, tag="ps1")
                nc.tensor.matmul(out=ps[:], lhsT=wsl(f"wihT{g}"),
                                 rhs=qe_sb[:], start=True, stop=True)
                nc.scalar.activation(out=xp[g][:], in_=ps[:], func=ACTF.Copy)
            gate = [spool.tile([100, 2], FP32, tag=f"gate{g}", name=f"gate{g}")
                    for g in range(4)]
            tmp1 = spool.tile([100, 2], FP32, tag="ltmp1")
            tmp2 = spool.tile([100, 2], FP32, tag="ltmp2")
            for t in range(Q):
                hin = h0[:] if t == 0 else hs[:, 2 * (t - 1):2 * t]
                for g in range(4):
                    ps = pp.tile([100, 2], FP32, tag="ps1")
                    nc.tensor.matmul(out=ps[:], lhsT=wsl(f"whhT{g}"),
                                     rhs=hin, start=True, stop=True)
                    nc.vector.scalar_tensor_tensor(
                        out=gate[g][:], in0=ps[:], scalar=0.0,
                        in1=xp[g][:, 2 * t:2 * t + 2],
                        op0=ALU.add, op1=ALU.add)
                for g, fn in ((0, ACTF.Sigmoid), (1, ACTF.Sigmoid),
                              (2, ACTF.Tanh), (3, ACTF.Sigmoid)):
                    nc.scalar.activation(out=gate[g][:], in_=gate[g][:],
                                         func=fn)
                nc.vector.tensor_tensor(out=tmp1[:], in0=gate[1][:],
                                        in1=ct[:], op=ALU.mult)
                nc.vector.tensor_tensor(out=tmp2[:], in0=gate[0][:],
                                        in1=gate[2][:], op=ALU.mult)
                nc.vector.tensor_tensor(out=ct[:], in0=tmp1[:], in1=tmp2[:],
                                        op=ALU.add)
                nc.scalar.activation(out=tmp1[:], in_=ct[:], func=ACTF.Tanh)
                nc.vector.tensor_tensor(out=hs[:, 2 * t:2 * t + 2],
                                        in0=gate[3][:], in1=tmp1[:],
                                        op=ALU.mult)
            qhs = cpool.tile([100, 2 * Q], FP32, tag="qhs")
            nc.vector.tensor_scalar_mul(qhs[:], hs[:], 1.0 / DIV)
            # qrel = [rel_lin_w^T @ qhs ; rel_lin_b @ qhs] as bf16 (101, 40):
            # sim_tile = (bf16 rel gather chunk).T @ qrel
            qrel = cpool.tile([101, 2 * Q], BF16, tag="qrel")
            psr = pp.tile([96, 2 * Q], FP32, tag="ps1")
            nc.tensor.matmul(out=psr[:], lhsT=wsl("rlwNT", cols=96),
                             rhs=qhs[:], start=True, stop=True)
            nc.scalar.activation(out=qrel[0:96, :], in_=psr[:],
                                 func=ACTF.Copy)
            roff = lay["rlwNT"][0]
            psr2 = pp.tile([5, 2 * Q], FP32, tag="ps1")
            nc.tensor.matmul(out=psr2[:], lhsT=W[0:100, roff + 96:roff + 101],
                             rhs=qhs[:], start=True, stop=True)
            nc.scalar.activation(out=qrel[96:101, :], in_=psr2[:],
                                 func=ACTF.Copy)

            # ---- shared big buffers (reused across examples)
            mbuf = bpool.tile([128, max(FPT, FPH)], FP32, tag="mbuf")
            f16 = bpool.tile([128, max(FPT, FPH)], BF16, tag="f16")
            G = bpool.tile([128, FPT], FP32, tag="G")
            stag = bpool.tile([128, 16 * TP], FP32, tag="stag")
            stag_d = dpool.tile([EP, TP], FP32, tag="stagd")
            LeA = cpool.tile([101, EP], FP32, tag="leA")
            LeB = cpool.tile([101, EP], FP32, tag="leB")
            # (1, EP) vectors packed into two 8-partition tiles (SBUF is
            # column-addressed; standalone 1-partition tiles waste 127/128)
            vecA = spool.tile([97, EP], FP32, tag="vecA", bufs=1)
            vecB = spool.tile([97, EP], FP32, tag="vecB", bufs=1)
            vecC = spool.tile([1, EP], FP32, tag="vecC", bufs=1)
            # 32-aligned bases; pager/esr at base 0 of different tiles so
            # tensor_tensor(pager, esr) has equal input bases; prr needs
            # base 0 too (wrap-matmul lhsT must match oh16 rhs base 0)
            pager = vecA[0:1, :]
            tprv = vecA[32:33, :]
            esr = vecB[0:1, :]
            esm = vecB[32:33, :]
            outsb = vecB[64:65, :]
            prr = vecC[0:1, :]
            f2e = wpool.tile([100, EP], FP32, tag="f2e")
            scat = wpool.tile([101, EP], FP32, tag="scat")
            Wt_t = spool.tile([128, T_T], FP32, tag="wtt", bufs=1)
            wpr = spool.tile([128, T_T], FP32, tag="wpr", bufs=1)
            qnode = spool.tile([101, 1], FP32, tag="qn", bufs=1)
            q2ec = spool.tile([101, 1], FP32, tag="q2ec", bufs=1)
            nc.gpsimd.memset(stag[:], 0.0)
            ooff = lay["ones100"][0]
            nc.sync.dma_start(out=LeB[100:101, :],
                              in_=le_d[100:101, 0:EP])
            nc.sync.dma_start(out=q2ec[100:101, 0:1],
                              in_=wimg_d[0:1, ooff:ooff + 1])
            nc.sync.dma_start(out=qnode[100:101, 0:1],
                              in_=wimg_d[0:1, ooff:ooff + 1])

            idx_base = [e * (2 * FPT + FPH) // 16 for e in (0, 1)]
            m_base = [e * (FPT + FPH) for e in (0, 1)]

            def attention(e, order, wt):
                T = T_T if order == "t" else T_H
                FPx = T * TP
                ib = idx_base[e] + (FPT // 16 if order == "t" else
                                    2 * FPT // 16)
                nc.gpsimd.dma_gather(
                    out_ap=f16[:, 0:FPx].rearrange("p (o f) -> p o f", o=1),
                    in_ap=rel16_d[:],
                    idxs_ap=midx[:, ib:ib + FPx // 16],
                    num_idxs=FPx, num_idxs_reg=FPx, elem_size=128,
                    transpose=True, single_packet=False)
                ssum = spool.tile([128, T], FP32, tag="ssum")
                wfn = spool.tile([128, T], FP32, tag="wfn")
                qsl = qrel[:].rearrange("p (t e) -> p t e", e=2)[:, :, e:e + 1]
                for t in range(T):
                    pss = pp.tile([128, Q], FP32, tag="ps1")
                    nc.tensor.matmul(out=pss[:],
                                     lhsT=f16[0:101, t * TP:(t + 1) * TP],
                                     rhs=qsl, start=True, stop=True)
                    es = spool.tile([128, Q], FP32, tag="es")
                    nc.scalar.activation(out=es[:], in_=pss[:], func=ACTF.Exp,
                                         accum_out=ssum[:, t:t + 1])
                    prod = spool.tile([128, Q], FP32, tag="prod")
                    nc.vector.scalar_tensor_tensor(
                        out=prod[:], in0=es[:], scalar=1.0, in1=pss[:],
                        op0=ALU.mult, op1=ALU.mult,
                        accum_out=wfn[:, t:t + 1])
                nc.vector.reciprocal(out=ssum[:], in_=ssum[:])
                nc.vector.tensor_tensor(out=wt[:, 0:T], in0=wfn[:],
                                        in1=ssum[:], op=ALU.mult)
                nc.scalar.activation(out=wt[:, 0:T], in_=wt[:, 0:T],
                                     func=ACTF.Exp)

            for e in (0, 1):
                # ---- per-example setup
                nc.sync.dma_start(out=LeA[0:101, :],
                                  in_=le_d[0:101, e * EP:(e + 1) * EP])
                nc.sync.dma_start(out=pager,
                                  in_=le_d[101:102, e * EP:(e + 1) * EP])
                nc.vector.tensor_copy(out=qnode[0:100, :],
                                      in_=hs[:, 2 * (Q - 1) + e:
                                             2 * (Q - 1) + e + 1])

                # head-order attention + e2f_softmax
                wth = spool.tile([128, T_H], FP32, tag="wth", bufs=1)
                attention(e, "h", wth)
                nc.gpsimd.dma_start(
                    out=mbuf[:, 0:FPH],
                    in_=mimg_d[:, m_base[e] + FPT:m_base[e] + FPT + FPH])
                esP = [pp4.tile([1, 512], FP32, tag="scat", name="esP")
                       for _ in range(4)]
                for b in range(4):
                    nc.tensor.matmul(out=esP[b][:], lhsT=zero_l[:, 0:1],
                                     rhs=zero_r[:], start=True, stop=True,
                                     skip_group_check=True)
                tpw_h = T_H // NWIN
                for t in range(T_H):
                    w = t // tpw_h
                    nc.tensor.matmul(
                        out=esP[w // 4][0:1,
                                        WINE * (w % 4):WINE * (w % 4 + 1)],
                        lhsT=wth[:, t:t + 1],
                        rhs=mbuf[:, t * TP:(t + 1) * TP],
                        start=False, stop=True, skip_group_check=True)
                for b in range(4):
                    nc.vector.tensor_scalar_max(
                        esm[:, 512 * b:512 * (b + 1)], esP[b][:], VERY_SMALL)
                nc.vector.reciprocal(out=esr, in_=esm)

                # tail-order attention (materializes fact) + load M
                attention(e, "t", Wt_t)
                nc.gpsimd.dma_start(out=mbuf[:, 0:FPT],
                                  in_=mimg_d[:, m_base[e]:m_base[e] + FPT])

                # ---- layers
                for i in range(L):
                    Le = LeA if i % 2 == 0 else LeB
                    Lenxt = LeB if i % 2 == 0 else LeA
                    psq = pp.tile([100, 1], FP32, tag="ps1")
                    nc.tensor.matmul(out=psq[:], lhsT=wsl(f"q2eT{i}"),
                                     rhs=qnode[:], start=True, stop=True)
                    nc.vector.tensor_copy(out=q2ec[0:100, :], in_=psq[:])
                    nc.vector.tensor_tensor(out=prr, in0=pager,
                                            in1=esr, op=ALU.mult)
                    for vec, col in ((pager, D), (prr, D + 1)):
                        psw = pp.tile([128, 16], FP32, tag="ps1")
                        for c in range(16):
                            nc.tensor.matmul(
                                out=psw[:], lhsT=vec[:, c * TP:(c + 1) * TP],
                                rhs=wsl("oh16")[0:1, 16 * c:16 * (c + 1)],
                                start=(c == 0), stop=(c == 15),
                                skip_group_check=True)
                        nc.vector.tensor_copy(
                            out=stag[:].rearrange(
                                "p (c j) -> p c j", j=TP)[:, :, col:col + 1],
                            in_=psw[:].rearrange("p (a b) -> p a b", b=1))
                    for c in range(16):
                        psh = pp.tile([128, D], FP32, tag="ps1")
                        nc.tensor.matmul(out=psh[:],
                                         lhsT=Le[:, c * TP:(c + 1) * TP],
                                         rhs=wsl(f"headT{i}"),
                                         start=True, stop=True)
                        nc.vector.tensor_copy(
                            out=stag[:, c * TP:c * TP + D], in_=psh[:])
                    nc.sync.dma_start(
                        out=stag_d[:].rearrange("(c p) j -> p c j", p=128),
                        in_=stag[:].rearrange("p (c j) -> p c j", j=TP))
                    nc.gpsimd.dma_gather(
                        out_ap=G[:].rearrange("p (t j) -> p t j", j=TP),
                        in_ap=stag_d[:],
                        idxs_ap=midx[:, idx_base[e]:idx_base[e] + FPT // 16],
                        num_idxs=FPT, num_idxs_reg=FPT, elem_size=128,
                        single_packet=False)
                    for t in range(T_T):
                        psa = pp.tile([128, D], FP32, tag="ps1")
                        nc.tensor.matmul(out=psa[:],
                                         lhsT=f16[0:101, t * TP:(t + 1) * TP],
                                         rhs=bimg[0:101, i * D:(i + 1) * D],
                                         start=True, stop=True)
                        gsl = G[:, t * TP:t * TP + D]
                        nc.vector.scalar_tensor_tensor(
                            out=gsl, in0=psa[:], scalar=0.0, in1=gsl,
                            op0=ALU.add, op1=ALU.add)
                    nc.vector.tensor_tensor(
                        out=wpr[:].rearrange("p (t j) -> p t j", j=1),
                        in0=Wt_t[:].rearrange("p (t j) -> p t j", j=1),
                        in1=G[:].rearrange(
                            "p (t j) -> p t j", j=TP)[:, :, D + 1:D + 2],
                        op=ALU.mult)
                    g3 = G[:].rearrange("p (t j) -> p t j", j=TP)
                    nc.vector.scalar_tensor_tensor(
                        out=g3[:, :, 0:D], in0=g3[:, :, 0:D], scalar=0.0,
                        in1=wpr[:].rearrange("p (t j) -> p t j", j=1)
                        .to_broadcast([128, T_T, D]),
                        op0=ALU.max, op1=ALU.mult)
                    scP = [pp4.tile([101, 512], FP32, tag="scat", name="scP")
                           for _ in range(4)]
                    for b in range(4):
                        nc.tensor.matmul(out=scP[b][:], lhsT=zero_l[:],
                                         rhs=zero_r[:], start=True, stop=True,
                                         skip_group_check=True)
                    tpw_t = T_T // NWIN
                    for t in range(T_T):
                        w = t // tpw_t
                        nc.tensor.matmul(
                            out=scP[w // 4][0:101,
                                            WINE * (w % 4):WINE * (w % 4 + 1)],
                            lhsT=G[:, t * TP:t * TP + 101],
                            rhs=mbuf[:, t * TP:(t + 1) * TP],
                            start=False, stop=True, skip_group_check=True)
                    for b in range(4):
                        nc.vector.tensor_copy(
                            out=scat[:, 512 * b:512 * (b + 1)],
                            in_=scP[b][:])
                    for c in range(4):
                        sl = slice(512 * c, 512 * (c + 1))
                        psf = pp.tile([100, 512], FP32, tag="ps1")
                        nc.tensor.matmul(out=psf[:], lhsT=wsl(f"selfT{i}"),
                                         rhs=Le[:, sl], start=True,
                                         stop=False, skip_group_check=True)
                        nc.tensor.matmul(out=psf[:], lhsT=wsl(f"tailT{i}"),
                                         rhs=scat[0:100, sl], start=False,
                                         stop=True, skip_group_check=True)
                        nc.scalar.activation(out=f2e[:, sl], in_=psf[:],
                                             func=ACTF.Relu)
                    nc.vector.tensor_scalar_mul(tprv, pager,
                                                1.0 - PAGERANK_LAMBDA)
                    for c in range(4):
                        sl = slice(512 * c, 512 * (c + 1))
                        psp = pp.tile([1, 512], FP32, tag="ps1")
                        nc.tensor.matmul(out=psp[:],
                                         lhsT=wsl("e100", rows=101),
                                         rhs=scat[0:101, sl],
                                         start=True, stop=True)
                        nc.vector.scalar_tensor_tensor(
                            out=pager[:, sl], in0=psp[:],
                            scalar=PAGERANK_LAMBDA, in1=tprv[:, sl],
                            op0=ALU.mult, op1=ALU.add)
                    s13 = spool.tile([100, 8], FP32, tag="s13")
                    scr = spool.tile([100, 512], FP32, tag="qscr", bufs=1)
                    for c in range(4):
                        sl = slice(512 * c, 512 * (c + 1))
                        psb = pp.tile([100, 512], FP32, tag="ps1")
                        nc.tensor.matmul(out=psb[:], lhsT=wsl("ones100"),
                                         rhs=pager[:, sl], start=True,
                                         stop=True)
                        nc.vector.scalar_tensor_tensor(
                            out=scr[:], in0=Le[0:100, sl], scalar=1.0,
                            in1=psb[:], op0=ALU.mult, op1=ALU.mult,
                            accum_out=s13[:, c:c + 1])
                        nc.vector.scalar_tensor_tensor(
                            out=scr[:], in0=f2e[:, sl], scalar=1.0,
                            in1=psb[:], op0=ALU.mult, op1=ALU.mult,
                            accum_out=s13[:, 4 + c:5 + c])
                    s1 = spool.tile([100, 2], FP32, tag="s1")
                    nc.vector.tensor_reduce(
                        out=s1[:],
                        in_=s13[:].rearrange("p (a b) -> p a b", b=4),
                        axis=mybir.AxisListType.X, op=ALU.add)
                    spr = spool.tile([1, 1], FP32, tag="spr")
                    nc.vector.tensor_reduce(out=spr[:], in_=pager,
                                            axis=mybir.AxisListType.X,
                                            op=ALU.add)
                    psq1 = pp.tile([100, 1], FP32, tag="ps1")
                    nc.tensor.matmul(out=psq1[:], lhsT=wsl(f"eAT{i}"),
                                     rhs=s1[:, 0:1], start=True, stop=False,
                                     skip_group_check=True)
                    nc.tensor.matmul(out=psq1[:], lhsT=wsl(f"eCT{i}"),
                                     rhs=s1[:, 1:2], start=False, stop=True,
                                     skip_group_check=True)
                    psq2 = pp.tile([100, 1], FP32, tag="ps1")
                    nc.tensor.matmul(out=psq2[:], lhsT=wsl(f"eBb{i}"),
                                     rhs=q2ec[:], start=True, stop=True)
                    psq3 = pp.tile([100, 1], FP32, tag="ps1")
                    nc.tensor.matmul(out=psq3[:], lhsT=wsl("ones100"),
                                     rhs=spr[:], start=True, stop=True)
                    sprb = spool.tile([100, 1], FP32, tag="sprb")
                    nc.vector.tensor_copy(out=sprb[:], in_=psq3[:])
                    tq = spool.tile([100, 1], FP32, tag="tq")
                    nc.vector.tensor_tensor(out=tq[:], in0=psq2[:],
                                            in1=sprb[:], op=ALU.mult)
                    nc.vector.tensor_tensor(out=qnode[0:100, :],
                                            in0=psq1[:], in1=tq[:],
                                            op=ALU.add)
                    psv = pp.tile([100, 1], FP32, tag="ps1")
                    nc.tensor.matmul(out=psv[:], lhsT=wsl(f"gBb{i}"),
                                     rhs=q2ec[:], start=True, stop=True)
                    biasc = spool.tile([100, 1], FP32, tag="biasc")
                    nc.vector.tensor_copy(out=biasc[:], in_=psv[:])
                    for c in range(4):
                        sl = slice(512 * c, 512 * (c + 1))
                        pse = pp.tile([100, 512], FP32, tag="ps1")
                        nc.tensor.matmul(out=pse[:], lhsT=wsl(f"gAT{i}"),
                                         rhs=Le[0:100, sl], start=True,
                                         stop=False, skip_group_check=True)
                        nc.tensor.matmul(out=pse[:], lhsT=wsl(f"gCT{i}"),
                                         rhs=f2e[:, sl], start=False,
                                         stop=True, skip_group_check=True)
                        nc.scalar.activation(out=Lenxt[0:100, sl],
                                             in_=pse[:], func=ACTF.Relu,
                                             bias=biasc[:])

                # ---- score
                Lefin = LeA if L % 2 == 0 else LeB
                for c in range(4):
                    sl = slice(512 * c, 512 * (c + 1))
                    pso = pp.tile([1, 512], FP32, tag="ps1")
                    nc.tensor.matmul(out=pso[:], lhsT=wsl("scoreT"),
                                     rhs=Lefin[:, sl], start=True,
                                     stop=True)
                    nc.vector.tensor_copy(out=outsb[:, sl], in_=pso[:])
                nc.sync.dma_start(out=out_d[e:e + 1, :], in_=outsb)

    # Raw Bass skips Bacc's codegen_inst_isa_subclasses; without it the
    # extended-inst InstISA subclasses (library reload) have empty bytes
    # and walrus fails with "ISA wrong length".
    from concourse.library_overlay import lower_extended_insts
    lower_extended_insts(nc)
    return nc



# ---------------------------------------------------------------- runner


def prepare(inputs):
    names = ["local_entity", "kb_fact_rel", "kb_head", "kb_tail",
             "query_text", "q2e_adj_mat"]
    w = {k: np.asarray(v, np.float32) for k, v in inputs.items()
         if k not in names}
    per = {n: np.asarray(inputs[n]) for n in names}
    B = per["local_entity"].shape[0]
    tpw_t = max(required_tpw(per["kb_tail"][b]) for b in range(B))
    tpw_h = max(required_tpw(per["kb_head"][b]) for b in range(B))
    T_T, T_H = NWIN * tpw_t, NWIN * tpw_h
    wimg = build_wimg(w)
    rel16 = build_rel16(w)
    bimg = build_bimg(w)
    exd = [build_example({n: per[n][b] for n in names}, w, tpw_t, tpw_h)
           for b in range(B)]
    in_maps = []
    for c in range(B // 2):
        im = build_core_inputs(exd[2 * c], exd[2 * c + 1], wimg, T_T, T_H)
        im["rel16"] = rel16
        im["bimg"] = bimg
        in_maps.append(im)
    e_mask = np.stack([e["e_mask"] for e in exd])
    return in_maps, T_T, T_H, rel16.shape, e_mask


_IN_NAMES = ["wimg", "mimg", "idx16", "leimg", "qemb2", "rel16", "bimg"]
_CACHE_DIR = os.environ.get(
    "BKERN_CACHE", os.path.expanduser("~/.cache/nnx58454504899045"))
_MEM = {}


def _export_path(key):
    return os.path.join(_CACHE_DIR, f"exp_{key}.bin")



def _install_cc_cache():
    """neuronx-cc hook + NEFF-bytes disk cache keyed on the HLO, so a fresh
    process skips the multi-second walrus compile."""
    from concourse.bass2jax import install_neuronx_cc_hook
    install_neuronx_cc_hook()
    import hashlib
    import libneuronxla
    if getattr(libneuronxla, "_bkern_cc_cached", False):
        return
    inner = libneuronxla.neuronx_cc

    def cached(code, code_format, platform_version, file_prefix):
        key = hashlib.sha256(b"bk1" + code).hexdigest()
        path = os.path.join(_CACHE_DIR, f"neff_{key}.bin")
        try:
            if os.path.exists(path):
                return 0, open(path, "rb").read()
        except Exception:
            pass
        r = inner(code, code_format, platform_version, file_prefix)
        try:
            if r[0] == 0 and isinstance(r[1], (bytes, bytearray)):
                os.makedirs(_CACHE_DIR, exist_ok=True)
                tmp = path + f".tmp{os.getpid()}"
                open(tmp, "wb").write(r[1])
                os.replace(tmp, path)
        except Exception:
            pass
        return r

    libneuronxla.neuronx_cc = cached
    libneuronxla._bkern_cc_cached = True


def _build_exported(T_T, T_H, rel16_shape, sample_maps):
    import jax
    from jax.sharding import Mesh, PartitionSpec
    from jax.experimental.shard_map import shard_map
    from concourse.bass2jax import (
        _bass_exec_p, partition_id_tensor, _fast_dispatch_active)
    _install_cc_cache()
    nc = build_nc(T_T, T_H, rel16_shape)
    pn = nc.partition_id_tensor.name if nc.partition_id_tensor else None
    out_name, out_shape = "out", (2, EP)
    out_avals = [jax.core.ShapedArray(out_shape, np.float32)]
    in_names_full = list(_IN_NAMES) + [out_name] + ([pn] if pn else [])

    def _body(*args):
        operands = list(args)
        if pn:
            operands.append(partition_id_tensor())
        outs = _bass_exec_p.bind(
            *operands, out_avals=tuple(out_avals),
            in_names=tuple(in_names_full), out_names=(out_name,),
            lowering_input_output_aliases=(), sim_require_finite=True,
            sim_require_nnan=True, nc=nc)
        return tuple(outs)

    devices = jax.devices()[:8]
    mesh = Mesh(np.asarray(devices), ("core",))
    nin = len(_IN_NAMES)
    f = jax.jit(
        shard_map(_body, mesh=mesh,
                  in_specs=(PartitionSpec("core"),) * (nin + 1),
                  out_specs=(PartitionSpec("core"),), check_rep=False),
        donate_argnums=(nin,), keep_unused=True)
    gl = [np.concatenate([m[n] for m in sample_maps], axis=0)
          for n in _IN_NAMES]
    zout = np.zeros((8 * 2, EP), np.float32)
    with _fast_dispatch_active(True):
        exported = jax.export.export(
            f, disabled_checks=[
                jax.export.DisabledSafetyCheck.custom_call("bass_exec")]
        )(*gl, zout)
    return exported


def _get_exported(T_T, T_H, rel16_shape, sample_maps):
    key = f"{T_T}_{T_H}_{rel16_shape[0]}_v6"
    if key in _MEM:
        return _MEM[key]
    import jax
    path = _export_path(key)
    exported = None
    if os.path.exists(path):
        try:
            _install_cc_cache()
            exported = jax.export.deserialize(open(path, "rb").read())
        except Exception:
            exported = None
    if exported is None:
        exported = _build_exported(T_T, T_H, rel16_shape, sample_maps)
        try:
            os.makedirs(_CACHE_DIR, exist_ok=True)
            tmp = path + ".tmp"
            open(tmp, "wb").write(exported.serialize())
            os.replace(tmp, path)
        except Exception:
            pass
    _MEM[key] = exported
    return exported


def run_device(in_maps, T_T, T_H, rel16_shape):
    import jax
    from jax.sharding import Mesh, PartitionSpec, NamedSharding
    exported = _get_exported(T_T, T_H, rel16_shape, in_maps)
    mesh = Mesh(np.asarray(jax.devices()[:8]), ("core",))
    sh = NamedSharding(mesh, PartitionSpec("core"))
    gl = [jax.device_put(np.concatenate([m[n] for m in in_maps], axis=0), sh)
          for n in _IN_NAMES]
    zout = jax.device_put(np.zeros((8 * 2, EP), np.float32), sh)
    callf = jax.jit(exported.call)
    res = callf(*gl, zout)
    return np.asarray(res[0])


def kernel(**inputs):
    in_maps, T_T, T_H, rel16_shape, e_mask = prepare(inputs)
    out = run_device(in_maps, T_T, T_H, rel16_shape)   # (16, EP)
    score = out[:, :E]
    return (score + (1.0 - e_mask) * VERY_NEG).astype(np.float32)


# revision 6
# speedup vs baseline: 1.1628x; 1.1119x over previous
"""Bass/Trainium2 kernel for nn_Exact_58454504899045 (GraftNet-style GNN).

Data-parallel over batch: 8 cores x 2 examples. One uniform SPMD program:
all data-dependent structure is normalized by padding facts into 128-fact
tiles aligned to 128-entity windows with a fixed tiles-per-window count.

Layouts:
  - entity tensors col-major (d on partitions, entity on free), EP=2048
  - fact tensors: col-major (101, FP) for matmul lhsT chunks; gathered/
    row-major tiles (128 facts, 128 cols) for DMA gather + merge matmuls
  - scatter by tail = per-tile merge matmul (lhsT = fact tile values,
    rhs = 0/1 merge block), accumulated in PSUM windows
  - gather by head = dma_gather of 512B staging rows from DRAM
    (staging row e = [hproj(e) | pagerank(e) | pr_ratio(e) | pad])
  - attention computed per 128-fact tile: sim matmul -> exp(ACT, accum sum)
    -> fused mul+reduce (DVE) giving wrapped (128, T) stats directly
  - e2f_softmax needs a scatter by head: separate head-sorted fact order
    with its own (cheap) attention pass + merge
"""
import os
import sys

sys.path.insert(0, "/opt/trn_rl_repo")

import numpy as np
import ml_dtypes

import concourse.bass as bass
import concourse.mybir as mybir
from concourse.tile import TileContext

FP32 = mybir.dt.float32
BF16 = mybir.dt.bfloat16
I16 = mybir.dt.int16
ALU = mybir.AluOpType
ACTF = mybir.ActivationFunctionType

NUM_ENTITY = 500000
NUM_RELATION = 6000
NUM_WORD = 200000
D = 100
L = 3
PAGERANK_LAMBDA = 0.8
FACT_SCALE = 3.0
VERY_NEG = -1e11
VERY_SMALL = 1e-10
E, F, Q = 2000, 8000, 20
EP = 2048
WINE = 128              # entity window (alignment of merge blocks)
NWIN = EP // WINE       # 16
TP = 128                # facts per tile
DIV = float(np.sqrt(D))

# ---------------------------------------------------------------- host side


def required_tpw(key_idx):
    """Max tiles (128 facts) needed by any 128-entity window."""
    cnt = np.bincount(key_idx // WINE, minlength=NWIN)
    return int(np.max((cnt + TP - 1) // TP))


def build_order(key_idx, tpw):
    """Facts sorted by key, packed into tpw tiles per 128-entity window.
    Returns slots (NWIN*tpw*128,) int32 orig fact index (-1 = pad)."""
    T = NWIN * tpw
    slots = np.full(T * TP, -1, np.int64)
    order = np.argsort(key_idx, kind="stable")
    k_s = key_idx[order]
    for w in range(NWIN):
        sel = order[(k_s // WINE) == w]
        base = w * tpw * TP
        assert len(sel) <= tpw * TP, "tiles-per-window overflow"
        slots[base:base + len(sel)] = sel
    return slots


def tloc_matrix(slots, key_idx, tpw):
    """(128, T) u8: window-local entity column of each slot (255 = pad).
    The one-hot merge blocks are built on device via iota+is_equal."""
    T = NWIN * tpw
    tl = np.full((TP, T), 255, np.uint8)
    pos = np.arange(T * TP)
    valid = slots >= 0
    p = pos % TP
    t = pos // TP
    c = key_idx[np.maximum(slots, 0)] - (t // tpw) * WINE
    tl[p[valid], t[valid]] = c[valid].astype(np.uint8)
    return tl


def wrap_idx(idx):
    """(N,) -> (128, N//16) int16, idx i at partition i%16 col i//16,
    replicated across the 8 gpsimd cores."""
    n = len(idx)
    assert n % 16 == 0
    a = np.ascontiguousarray(idx.astype(np.int16).reshape(n // 16, 16).T)
    return np.tile(a, (8, 1))


def _wimg_entries():
    """Static weight-image layout: list of (name, rows, cols)."""
    ent = []
    for i in range(L):
        ent += [(f"headT{i}", 101, D), (f"selfT{i}", 101, D),
                (f"tailT{i}", 100, D), (f"gAT{i}", 100, D),
                (f"gCT{i}", 100, D), (f"gBb{i}", 101, D),
                (f"eAT{i}", 100, D), (f"eCT{i}", 100, D),
                (f"eBb{i}", 101, D), (f"q2eT{i}", 101, D)]
    for g in range(4):
        ent += [(f"wihT{g}", 101, D), (f"whhT{g}", 100, D)]
    ent += [("scoreT", 101, 1), ("ones100", 1, D), ("eye16", 16, 16),
            ("zeros512", 1, 512), ("oh16", 1, 256),
            ("rlwNT", 100, D), ("rlb", 100, 1), ("e100", 128, 1),
            ("iota", 128, TP)]
    return ent


def wimg_layout():
    off, lay = 0, {}
    for name, rows, cols in _wimg_entries():
        lay[name] = (off, rows, cols)
        off += cols
    total = max(off, 512)
    return lay, total


def build_wimg(w):
    lay, total = wimg_layout()
    img = np.zeros((128, total), np.float32)

    def put(name, arr):
        off, rows, cols = lay[name]
        assert arr.shape == (rows, cols), (name, arr.shape)
        img[:rows, off:off + cols] = arr

    for i in range(L):
        # self_b folded here: the gathered staging rows supply self_b to e2f
        put(f"headT{i}", np.vstack([w["head_w"][i].T,
                                    (w["head_b"][i] + w["self_b"][i])[None]]))
        put(f"selfT{i}", np.vstack([w["self_w"][i].T, w["self_b"][i][None]]))
        put(f"tailT{i}", FACT_SCALE * w["tail_w"][i].T)
        gA, gB, gC = (w["e2e_w"][i][:, :D], w["e2e_w"][i][:, D:2 * D],
                      w["e2e_w"][i][:, 2 * D:])
        eA, eB, eC = (w["e2q_w"][i][:, :D], w["e2q_w"][i][:, D:2 * D],
                      w["e2q_w"][i][:, 2 * D:])
        put(f"gAT{i}", gA.T)
        put(f"gCT{i}", gC.T)
        put(f"gBb{i}", np.vstack([gB.T, w["e2e_b"][i][None]]))
        put(f"eAT{i}", eA.T)
        put(f"eCT{i}", eC.T)
        put(f"eBb{i}", np.vstack([eB.T, w["e2q_b"][i][None]]))
        put(f"q2eT{i}", np.vstack([w["q2e_w"][i].T, w["q2e_b"][i][None]]))
    bias = w["lstm_bih"] + w["lstm_bhh"]
    for g in range(4):
        sl = slice(g * D, (g + 1) * D)
        put(f"wihT{g}", np.vstack([w["lstm_wih"][sl].T, bias[sl][None]]))
        put(f"whhT{g}", w["lstm_whh"][sl].T)
    put("scoreT", np.vstack([w["score_w"].T, w["score_b"][None]]))
    put("ones100", np.ones((1, D), np.float32))
    put("eye16", np.eye(16, dtype=np.float32))
    oh = np.zeros((1, 256), np.float32)
    oh[0, np.arange(16) * 16 + np.arange(16)] = 1.0
    put("oh16", oh)
    put("rlwNT", w["rel_lin_w"].astype(np.float32))
    put("rlb", w["rel_lin_b"][:, None].astype(np.float32))
    e100 = np.zeros((128, 1), np.float32)
    e100[100, 0] = 1.0
    put("e100", e100)
    put("iota", np.tile(np.arange(TP, dtype=np.float32), (128, 1)))
    return img


def build_rel16(w):
    """(6016, 128) bf16: row r = rel_table[r] | 1.0 | zeros."""
    img = np.zeros((6016, 128), ml_dtypes.bfloat16)
    img[:NUM_RELATION + 1, :D] = w["rel_table"].astype(ml_dtypes.bfloat16)
    img[:NUM_RELATION + 1, D] = 1.0
    return img


def build_bimg(w):
    """bf16 (128, L*D): selfR{i} = [(self_w@rel_lin_w).T; (self_w@rel_lin_b)]
    so A = raw_rel_gather @ selfR directly (fact_emb never materialized)."""
    img = np.zeros((128, L * D), ml_dtypes.bfloat16)
    for i in range(L):
        sr = (w["self_w"][i] @ w["rel_lin_w"]).T
        img[:D, i * D:(i + 1) * D] = sr.astype(ml_dtypes.bfloat16)
        img[D, i * D:(i + 1) * D] = (
            w["self_w"][i] @ w["rel_lin_b"]).astype(ml_dtypes.bfloat16)
    return img


def build_example(exi, w, tpw_t, tpw_h):
    """Host data for one example."""
    head, tail, rel = exi["kb_head"], exi["kb_tail"], exi["kb_fact_rel"]
    T_T, T_H = NWIN * tpw_t, NWIN * tpw_h
    slots_t = build_order(tail, tpw_t)
    slots_h = build_order(head, tpw_h)
    M = tloc_matrix(slots_t, tail, tpw_t)
    Mh = tloc_matrix(slots_h, head, tpw_h)
    rel_t = np.where(slots_t >= 0, rel[np.maximum(slots_t, 0)], NUM_RELATION)
    rel_h = np.where(slots_h >= 0, rel[np.maximum(slots_h, 0)], NUM_RELATION)
    head_t = np.where(slots_t >= 0, head[np.maximum(slots_t, 0)], EP - 1)

    le = np.zeros((102, EP), np.float32)
    le[:D, :E] = w["entity_table"][exi["local_entity"]].T
    le[D] = 1.0
    le[D + 1, :E] = exi["q2e_adj_mat"][:, 0]

    qe = np.zeros((101, Q), np.float32)
    qe[:D] = w["word_table"][exi["query_text"]].T
    qe[D] = 1.0

    return dict(M=M, Mh=Mh, idx_head=wrap_idx(head_t),
                idx_relt=wrap_idx(rel_t), idx_relh=wrap_idx(rel_h),
                le=le, qemb=qe,
                e_mask=(exi["local_entity"] != NUM_ENTITY).astype(np.float32),
                q_mask=(exi["query_text"] != NUM_WORD).astype(np.float32))


def build_core_inputs(ex0, ex1, wimg, T_T, T_H):
    m = np.concatenate([np.concatenate([e["M"], e["Mh"]], axis=1)
                        for e in (ex0, ex1)], axis=1)
    idx = np.concatenate([np.concatenate(
        [e["idx_head"], e["idx_relt"], e["idx_relh"]], axis=1)
        for e in (ex0, ex1)], axis=1)
    le = np.concatenate([ex0["le"], ex1["le"]], axis=1)
    qe = np.zeros((101, 2 * Q), np.float32)
    qe[:, 0::2] = ex0["qemb"]
    qe[:, 1::2] = ex1["qemb"]
    return {"wimg": wimg, "mimg": np.ascontiguousarray(m),
            "idx16": np.ascontiguousarray(idx),
            "leimg": np.ascontiguousarray(le),
            "qemb2": np.ascontiguousarray(qe)}


# ---------------------------------------------------------------- device side


def build_nc(T_T, T_H, rel16_shape):
    FPT, FPH = T_T * TP, T_H * TP
    lay, wtot = wimg_layout()
    nc = bass.Bass("TRN2")

    wimg_d = nc.dram_tensor("wimg", [128, wtot], FP32, kind="ExternalInput")
    mimg_d = nc.dram_tensor("mimg", [128, 2 * (T_T + T_H)], mybir.dt.uint8,
                            kind="ExternalInput")
    idx_d = nc.dram_tensor("idx16", [128, 2 * (2 * FPT + FPH) // 16], I16,
                           kind="ExternalInput")
    le_d = nc.dram_tensor("leimg", [102, 2 * EP], FP32, kind="ExternalInput")
    qe_d = nc.dram_tensor("qemb2", [101, 2 * Q], FP32, kind="ExternalInput")
    rel16_d = nc.dram_tensor("rel16", list(rel16_shape), BF16,
                             kind="ExternalInput")
    bimg_d = nc.dram_tensor("bimg", [128, L * D], BF16, kind="ExternalInput")
    out_d = nc.dram_tensor("out", [2, EP], FP32, kind="ExternalOutput")

    # dma_gather is an extended GPSIMD instruction: load its ucode library
    # in the pre-Tile preamble so it precedes every gather.
    from concourse import library_config
    nc.gpsimd.load_library(library_config.mlp)

    with TileContext(nc) as tc:
        with (tc.tile_pool(name="const", bufs=1) as cpool,
              tc.tile_pool(name="big", bufs=1) as bpool,
              tc.tile_pool(name="work", bufs=1) as wpool,
              tc.tile_pool(name="small", bufs=2) as spool,
              tc.tile_pool(name="psum", bufs=3, space="PSUM") as pp,
              tc.tile_pool(name="psum4", bufs=4, space="PSUM") as pp4,
              tc.tile_pool(name="dram", bufs=1, space="DRAM") as dpool):

            # ---- constants
            W = cpool.tile([128, wtot], FP32, tag="wimg")
            nc.sync.dma_start(out=W[:], in_=wimg_d[:])

            def wsl(name, rows=None, cols=None):
                off, r, c = lay[name]
                return W[0:(rows or r), off:off + (cols or c)]

            zoff = lay["zeros512"][0]
            zero_l = W[0:1, zoff:zoff + 101]
            zero_r = W[0:1, zoff:zoff + 512]

            bimg = cpool.tile([128, L * D], BF16, tag="bimg")
            nc.sync.dma_start(out=bimg[:], in_=bimg_d[:])
            midx = cpool.tile([128, 2 * (2 * FPT + FPH) // 16], I16,
                              tag="midx")
            nc.sync.dma_start(out=midx[:], in_=idx_d[:])
            qe_sb = cpool.tile([101, 2 * Q], FP32, tag="qesb")
            nc.sync.dma_start(out=qe_sb[:], in_=qe_d[:])
            # window-local merge columns, cast once to f32 for is_equal
            tloc8 = cpool.tile([128, 2 * (T_T + T_H)], mybir.dt.uint8,
                               tag="tloc8")
            nc.sync.dma_start(out=tloc8[:], in_=mimg_d[:])
            tlocf = cpool.tile([128, 2 * (T_T + T_H)], FP32, tag="tlocf")
            nc.vector.tensor_copy(out=tlocf[:], in_=tloc8[:])

            # ---- LSTM over both examples (interleaved t-major): hs (100,40)
            hs = cpool.tile([100, 2 * Q], FP32, tag="hs")
            ct = spool.tile([100, 2], FP32, tag="lstm_c")
            h0 = spool.tile([100, 2], FP32, tag="lstm_h0")
            nc.gpsimd.memset(ct[:], 0.0)
            nc.gpsimd.memset(h0[:], 0.0)
            xp = [cpool.tile([100, 2 * Q], FP32, tag=f"xp{g}", name=f"xp{g}")
                  for g in range(4)]
            for g in range(4):
                ps = pp.tile([100, 2 * Q], FP32, tag="ps1")
                nc.tensor.matmul(out=ps[:], lhsT=wsl(f"wihT{g}"),
                                 rhs=qe_sb[:], start=True, stop=True)
                nc.scalar.activation(out=xp[g][:], in_=ps[:], func=ACTF.Copy)
            gate = [spool.tile([100, 2], FP32, tag=f"gate{g}", name=f"gate{g}")
                    for g in range(4)]
            tmp1 = spool.tile([100, 2], FP32, tag="ltmp1")
            tmp2 = spool.tile([100, 2], FP32, tag="ltmp2")
            for t in range(Q):
                hin = h0[:] if t == 0 else hs[:, 2 * (t - 1):2 * t]
                for g in range(4):
                    ps = pp.tile([100, 2], FP32, tag="ps1")
                    nc.tensor.matmul(out=ps[:], lhsT=wsl(f"whhT{g}"),
                                     rhs=hin, start=True, stop=True)
                    nc.vector.scalar_tensor_tensor(
                        out=gate[g][:], in0=ps[:], scalar=0.0,
                        in1=xp[g][:, 2 * t:2 * t + 2],
                        op0=ALU.add, op1=ALU.add)
                for g, fn in ((0, ACTF.Sigmoid), (1, ACTF.Sigmoid),
                              (2, ACTF.Tanh), (3, ACTF.Sigmoid)):
                    nc.scalar.activation(out=gate[g][:], in_=gate[g][:],
                                         func=fn)
                nc.vector.tensor_tensor(out=tmp1[:], in0=gate[1][:],
                                        in1=ct[:], op=ALU.mult)
                nc.vector.tensor_tensor(out=tmp2[:], in0=gate[0][:],
                                        in1=gate[2][:], op=ALU.mult)
                nc.vector.tensor_tensor(out=ct[:], in0=tmp1[:], in1=tmp2[:],
                                        op=ALU.add)
                nc.scalar.activation(out=tmp1[:], in_=ct[:], func=ACTF.Tanh)
                nc.vector.tensor_tensor(out=hs[:, 2 * t:2 * t + 2],
                                        in0=gate[3][:], in1=tmp1[:],
                                        op=ALU.mult)
            qhs = cpool.tile([100, 2 * Q], FP32, tag="qhs")
            nc.vector.tensor_scalar_mul(qhs[:], hs[:], 1.0 / DIV)
            # qrel = [rel_lin_w^T @ qhs ; rel_lin_b @ qhs] as bf16 (101, 40):
            # sim_tile = (bf16 rel gather chunk).T @ qrel
            qrel = cpool.tile([101, 2 * Q], BF16, tag="qrel")
            psr = pp.tile([96, 2 * Q], FP32, tag="ps1")
            nc.tensor.matmul(out=psr[:], lhsT=wsl("rlwNT", cols=96),
                             rhs=qhs[:], start=True, stop=True)
            nc.scalar.activation(out=qrel[0:96, :], in_=psr[:],
                                 func=ACTF.Copy)
            roff = lay["rlwNT"][0]
            psr2 = pp.tile([5, 2 * Q], FP32, tag="ps1")
            nc.tensor.matmul(out=psr2[:], lhsT=W[0:100, roff + 96:roff + 101],
                             rhs=qhs[:], start=True, stop=True)
            nc.scalar.activation(out=qrel[96:101, :], in_=psr2[:],
                                 func=ACTF.Copy)

            # ---- shared big buffers (reused across examples)
            mbuf = bpool.tile([128, max(FPT, FPH)], FP32, tag="mbuf")
            f16 = bpool.tile([128, max(FPT, FPH)], BF16, tag="f16")
            G = bpool.tile([128, FPT], FP32, tag="G")
            stag = bpool.tile([128, 16 * TP], FP32, tag="stag")
            stag_d = dpool.tile([EP, TP], FP32, tag="stagd")
            LeA = cpool.tile([101, EP], FP32, tag="leA")
            LeB = cpool.tile([101, EP], FP32, tag="leB")
            # (1, EP) vectors packed into two 8-partition tiles (SBUF is
            # column-addressed; standalone 1-partition tiles waste 127/128)
            vecA = spool.tile([97, EP], FP32, tag="vecA", bufs=1)
            vecB = spool.tile([97, EP], FP32, tag="vecB", bufs=1)
            vecC = spool.tile([1, EP], FP32, tag="vecC", bufs=1)
            # 32-aligned bases; pager/esr at base 0 of different tiles so
            # tensor_tensor(pager, esr) has equal input bases; prr needs
            # base 0 too (wrap-matmul lhsT must match oh16 rhs base 0)
            pager = vecA[0:1, :]
            tprv = vecA[32:33, :]
            esr = vecB[0:1, :]
            esm = vecB[32:33, :]
            outsb = vecB[64:65, :]
            prr = vecC[0:1, :]
            f2e = wpool.tile([100, EP], FP32, tag="f2e")
            scat = wpool.tile([101, EP], FP32, tag="scat")
            Wt_t = spool.tile([128, T_T], FP32, tag="wtt", bufs=1)
            wpr = spool.tile([128, T_T], FP32, tag="wpr", bufs=1)
            qnode = spool.tile([101, 1], FP32, tag="qn", bufs=1)
            q2ec = spool.tile([101, 1], FP32, tag="q2ec", bufs=1)
            nc.gpsimd.memset(stag[:], 0.0)
            ooff = lay["ones100"][0]
            nc.sync.dma_start(out=LeB[100:101, :],
                              in_=le_d[100:101, 0:EP])
            nc.sync.dma_start(out=q2ec[100:101, 0:1],
                              in_=wimg_d[0:1, ooff:ooff + 1])
            nc.sync.dma_start(out=qnode[100:101, 0:1],
                              in_=wimg_d[0:1, ooff:ooff + 1])

            idx_base = [e * (2 * FPT + FPH) // 16 for e in (0, 1)]
            m_base = [e * (FPT + FPH) for e in (0, 1)]

            def attention(e, order, wt):
                T = T_T if order == "t" else T_H
                FPx = T * TP
                ib = idx_base[e] + (FPT // 16 if order == "t" else
                                    2 * FPT // 16)
                nc.gpsimd.dma_gather(
                    out_ap=f16[:, 0:FPx].rearrange("p (o f) -> p o f", o=1),
                    in_ap=rel16_d[:],
                    idxs_ap=midx[:, ib:ib + FPx // 16],
                    num_idxs=FPx, num_idxs_reg=FPx, elem_size=128,
                    transpose=True, single_packet=False)
                ssum = spool.tile([128, T], FP32, tag="ssum")
                wfn = spool.tile([128, T], FP32, tag="wfn")
                qsl = qrel[:].rearrange("p (t e) -> p t e", e=2)[:, :, e:e + 1]
                for t in range(T):
                    pss = pp.tile([128, Q], FP32, tag="ps1")
                    nc.tensor.matmul(out=pss[:],
                                     lhsT=f16[0:101, t * TP:(t + 1) * TP],
                                     rhs=qsl, start=True, stop=True)
                    es = spool.tile([128, Q], FP32, tag="es")
                    nc.scalar.activation(out=es[:], in_=pss[:], func=ACTF.Exp,
                                         accum_out=ssum[:, t:t + 1])
                    prod = spool.tile([128, Q], FP32, tag="prod")
                    nc.vector.scalar_tensor_tensor(
                        out=prod[:], in0=es[:], scalar=1.0, in1=pss[:],
                        op0=ALU.mult, op1=ALU.mult,
                        accum_out=wfn[:, t:t + 1])
                nc.vector.reciprocal(out=ssum[:], in_=ssum[:])
                nc.vector.tensor_tensor(out=wt[:, 0:T], in0=wfn[:],
                                        in1=ssum[:], op=ALU.mult)
                nc.scalar.activation(out=wt[:, 0:T], in_=wt[:, 0:T],
                                     func=ACTF.Exp)

            for e in (0, 1):
                # ---- per-example setup
                nc.sync.dma_start(out=LeA[0:101, :],
                                  in_=le_d[0:101, e * EP:(e + 1) * EP])
                nc.sync.dma_start(out=pager,
                                  in_=le_d[101:102, e * EP:(e + 1) * EP])
                nc.vector.tensor_copy(out=qnode[0:100, :],
                                      in_=hs[:, 2 * (Q - 1) + e:
                                             2 * (Q - 1) + e + 1])

                # head-order attention + e2f_softmax
                wth = spool.tile([128, T_H], FP32, tag="wth", bufs=1)
                attention(e, "h", wth)
                hb = e * (T_T + T_H) + T_T
                for t in range(T_H):
                    nc.vector.tensor_scalar(
                        mbuf[:, t * TP:(t + 1) * TP], wsl("iota"),
                        tlocf[:, hb + t:hb + t + 1], None, ALU.is_equal)
                esP = [pp4.tile([1, 512], FP32, tag="scat", name="esP")
                       for _ in range(4)]
                for b in range(4):
                    nc.tensor.matmul(out=esP[b][:], lhsT=zero_l[:, 0:1],
                                     rhs=zero_r[:], start=True, stop=True,
                                     skip_group_check=True)
                tpw_h = T_H // NWIN
                for t in range(T_H):
                    w = t // tpw_h
                    nc.tensor.matmul(
                        out=esP[w // 4][0:1,
                                        WINE * (w % 4):WINE * (w % 4 + 1)],
                        lhsT=wth[:, t:t + 1],
                        rhs=mbuf[:, t * TP:(t + 1) * TP],
                        start=False, stop=True, skip_group_check=True)
                for b in range(4):
                    nc.vector.tensor_scalar_max(
                        esm[:, 512 * b:512 * (b + 1)], esP[b][:], VERY_SMALL)
                nc.vector.reciprocal(out=esr, in_=esm)

                # tail-order attention (materializes fact) + load M
                attention(e, "t", Wt_t)
                tb = e * (T_T + T_H)
                for t in range(T_T):
                    nc.vector.tensor_scalar(
                        mbuf[:, t * TP:(t + 1) * TP], wsl("iota"),
                        tlocf[:, tb + t:tb + t + 1], None, ALU.is_equal)

                # ---- layers
                for i in range(L):
                    Le = LeA if i % 2 == 0 else LeB
                    Lenxt = LeB if i % 2 == 0 else LeA
                    psq = pp.tile([100, 1], FP32, tag="ps1")
                    nc.tensor.matmul(out=psq[:], lhsT=wsl(f"q2eT{i}"),
                                     rhs=qnode[:], start=True, stop=True)
                    nc.vector.tensor_copy(out=q2ec[0:100, :], in_=psq[:])
                    nc.vector.tensor_tensor(out=prr, in0=pager,
                                            in1=esr, op=ALU.mult)
                    for vec, col in ((pager, D), (prr, D + 1)):
                        psw = pp.tile([128, 16], FP32, tag="ps1")
                        for c in range(16):
                            nc.tensor.matmul(
                                out=psw[:], lhsT=vec[:, c * TP:(c + 1) * TP],
                                rhs=wsl("oh16")[0:1, 16 * c:16 * (c + 1)],
                                start=(c == 0), stop=(c == 15),
                                skip_group_check=True)
                        nc.vector.tensor_copy(
                            out=stag[:].rearrange(
                                "p (c j) -> p c j", j=TP)[:, :, col:col + 1],
                            in_=psw[:].rearrange("p (a b) -> p a b", b=1))
                    for c in range(16):
                        psh = pp.tile([128, D], FP32, tag="ps1")
                        nc.tensor.matmul(out=psh[:],
                                         lhsT=Le[:, c * TP:(c + 1) * TP],
                                         rhs=wsl(f"headT{i}"),
                                         start=True, stop=True)
                        nc.vector.tensor_copy(
                            out=stag[:, c * TP:c * TP + D], in_=psh[:])
                    nc.sync.dma_start(
                        out=stag_d[:].rearrange("(c p) j -> p c j", p=128),
                        in_=stag[:].rearrange("p (c j) -> p c j", j=TP))
                    nc.gpsimd.dma_gather(
                        out_ap=G[:].rearrange("p (t j) -> p t j", j=TP),
                        in_ap=stag_d[:],
                        idxs_ap=midx[:, idx_base[e]:idx_base[e] + FPT // 16],
                        num_idxs=FPT, num_idxs_reg=FPT, elem_size=128,
                        single_packet=False)
                    for t in range(T_T):
                        psa = pp.tile([128, D], FP32, tag="ps1")
                        nc.tensor.matmul(out=psa[:],
                                         lhsT=f16[0:101, t * TP:(t + 1) * TP],
                                         rhs=bimg[0:101, i * D:(i + 1) * D],
                                         start=True, stop=True)
                        gsl = G[:, t * TP:t * TP + D]
                        nc.vector.scalar_tensor_tensor(
                            out=gsl, in0=psa[:], scalar=0.0, in1=gsl,
                            op0=ALU.add, op1=ALU.add)
                    nc.vector.tensor_tensor(
                        out=wpr[:].rearrange("p (t j) -> p t j", j=1),
                        in0=Wt_t[:].rearrange("p (t j) -> p t j", j=1),
                        in1=G[:].rearrange(
                            "p (t j) -> p t j", j=TP)[:, :, D + 1:D + 2],
                        op=ALU.mult)
                    g3 = G[:].rearrange("p (t j) -> p t j", j=TP)
                    nc.vector.scalar_tensor_tensor(
                        out=g3[:, :, 0:D], in0=g3[:, :, 0:D], scalar=0.0,
                        in1=wpr[:].rearrange("p (t j) -> p t j", j=1)
                        .to_broadcast([128, T_T, D]),
                        op0=ALU.max, op1=ALU.mult)
                    scP = [pp4.tile([101, 512], FP32, tag="scat", name="scP")
                           for _ in range(4)]
                    for b in range(4):
                        nc.tensor.matmul(out=scP[b][:], lhsT=zero_l[:],
                                         rhs=zero_r[:], start=True, stop=True,
                                         skip_group_check=True)
                    tpw_t = T_T // NWIN
                    for t in range(T_T):
                        w = t // tpw_t
                        nc.tensor.matmul(
                            out=scP[w // 4][0:101,
                                            WINE * (w % 4):WINE * (w % 4 + 1)],
                            lhsT=G[:, t * TP:t * TP + 101],
                            rhs=mbuf[:, t * TP:(t + 1) * TP],
                            start=False, stop=True, skip_group_check=True)
                    for b in range(4):
                        nc.vector.tensor_copy(
                            out=scat[:, 512 * b:512 * (b + 1)],
                            in_=scP[b][:])
                    for c in range(4):
                        sl = slice(512 * c, 512 * (c + 1))
                        psf = pp.tile([100, 512], FP32, tag="ps1")
                        nc.tensor.matmul(out=psf[:], lhsT=wsl(f"selfT{i}"),
                                         rhs=Le[:, sl], start=True,
                                         stop=False, skip_group_check=True)
                        nc.tensor.matmul(out=psf[:], lhsT=wsl(f"tailT{i}"),
                                         rhs=scat[0:100, sl], start=False,
                                         stop=True, skip_group_check=True)
                        nc.scalar.activation(out=f2e[:, sl], in_=psf[:],
                                             func=ACTF.Relu)
                    nc.vector.tensor_scalar_mul(tprv, pager,
                                                1.0 - PAGERANK_LAMBDA)
                    for c in range(4):
                        sl = slice(512 * c, 512 * (c + 1))
                        psp = pp.tile([1, 512], FP32, tag="ps1")
                        nc.tensor.matmul(out=psp[:],
                                         lhsT=wsl("e100", rows=101),
                                         rhs=scat[0:101, sl],
                                         start=True, stop=True)
                        nc.vector.scalar_tensor_tensor(
                            out=pager[:, sl], in0=psp[:],
                            scalar=PAGERANK_LAMBDA, in1=tprv[:, sl],
                            op0=ALU.mult, op1=ALU.add)
                    s13 = spool.tile([100, 8], FP32, tag="s13")
                    scr = spool.tile([100, 512], FP32, tag="qscr", bufs=1)
                    for c in range(4):
                        sl = slice(512 * c, 512 * (c + 1))
                        psb = pp.tile([100, 512], FP32, tag="ps1")
                        nc.tensor.matmul(out=psb[:], lhsT=wsl("ones100"),
                                         rhs=pager[:, sl], start=True,
                                         stop=True)
                        nc.vector.scalar_tensor_tensor(
                            out=scr[:], in0=Le[0:100, sl], scalar=1.0,
                            in1=psb[:], op0=ALU.mult, op1=ALU.mult,
                            accum_out=s13[:, c:c + 1])
                        nc.vector.scalar_tensor_tensor(
                            out=scr[:], in0=f2e[:, sl], scalar=1.0,
                            in1=psb[:], op0=ALU.mult, op1=ALU.mult,
                            accum_out=s13[:, 4 + c:5 + c])
                    s1 = spool.tile([100, 2], FP32, tag="s1")
                    nc.vector.tensor_reduce(
                        out=s1[:],
                        in_=s13[:].rearrange("p (a b) -> p a b", b=4),
                        axis=mybir.AxisListType.X, op=ALU.add)
                    spr = spool.tile([1, 1], FP32, tag="spr")
                    nc.vector.tensor_reduce(out=spr[:], in_=pager,
                                            axis=mybir.AxisListType.X,
                                            op=ALU.add)
                    psq1 = pp.tile([100, 1], FP32, tag="ps1")
                    nc.tensor.matmul(out=psq1[:], lhsT=wsl(f"eAT{i}"),
                                     rhs=s1[:, 0:1], start=True, stop=False,
                                     skip_group_check=True)
                    nc.tensor.matmul(out=psq1[:], lhsT=wsl(f"eCT{i}"),
                                     rhs=s1[:, 1:2], start=False, stop=True,
                                     skip_group_check=True)
                    psq2 = pp.tile([100, 1], FP32, tag="ps1")
                    nc.tensor.matmul(out=psq2[:], lhsT=wsl(f"eBb{i}"),
                                     rhs=q2ec[:], start=True, stop=True)
                    psq3 = pp.tile([100, 1], FP32, tag="ps1")
                    nc.tensor.matmul(out=psq3[:], lhsT=wsl("ones100"),
                                     rhs=spr[:], start=True, stop=True)
                    sprb = spool.tile([100, 1], FP32, tag="sprb")
                    nc.vector.tensor_copy(out=sprb[:], in_=psq3[:])
                    tq = spool.tile([100, 1], FP32, tag="tq")
                    nc.vector.tensor_tensor(out=tq[:], in0=psq2[:],
                                            in1=sprb[:], op=ALU.mult)
                    nc.vector.tensor_tensor(out=qnode[0:100, :],
                                            in0=psq1[:], in1=tq[:],
                                            op=ALU.add)
                    psv = pp.tile([100, 1], FP32, tag="ps1")
                    nc.tensor.matmul(out=psv[:], lhsT=wsl(f"gBb{i}"),
                                     rhs=q2ec[:], start=True, stop=True)
                    biasc = spool.tile([100, 1], FP32, tag="biasc")
                    nc.vector.tensor_copy(out=biasc[:], in_=psv[:])
                    for c in range(4):
                        sl = slice(512 * c, 512 * (c + 1))
                        pse = pp.tile([100, 512], FP32, tag="ps1")
                        nc.tensor.matmul(out=pse[:], lhsT=wsl(f"gAT{i}"),
                                         rhs=Le[0:100, sl], start=True,
                                         stop=False, skip_group_check=True)
                        nc.tensor.matmul(out=pse[:], lhsT=wsl(f"gCT{i}"),
                                         rhs=f2e[:, sl], start=False,
                                         stop=True, skip_group_check=True)
                        nc.scalar.activation(out=Lenxt[0:100, sl],
                                             in_=pse[:], func=ACTF.Relu,
                                             bias=biasc[:])

                # ---- score
                Lefin = LeA if L % 2 == 0 else LeB
                for c in range(4):
                    sl = slice(512 * c, 512 * (c + 1))
                    pso = pp.tile([1, 512], FP32, tag="ps1")
                    nc.tensor.matmul(out=pso[:], lhsT=wsl("scoreT"),
                                     rhs=Lefin[:, sl], start=True,
                                     stop=True)
                    nc.vector.tensor_copy(out=outsb[:, sl], in_=pso[:])
                nc.sync.dma_start(out=out_d[e:e + 1, :], in_=outsb)

    # Raw Bass skips Bacc's codegen_inst_isa_subclasses; without it the
    # extended-inst InstISA subclasses (library reload) have empty bytes
    # and walrus fails with "ISA wrong length".
    from concourse.library_overlay import lower_extended_insts
    lower_extended_insts(nc)
    return nc



# ---------------------------------------------------------------- runner


def prepare(inputs):
    names = ["local_entity", "kb_fact_rel", "kb_head", "kb_tail",
             "query_text", "q2e_adj_mat"]
    w = {k: np.asarray(v, np.float32) for k, v in inputs.items()
         if k not in names}
    per = {n: np.asarray(inputs[n]) for n in names}
    B = per["local_entity"].shape[0]
    tpw_t = max(required_tpw(per["kb_tail"][b]) for b in range(B))
    tpw_h = max(required_tpw(per["kb_head"][b]) for b in range(B))
    T_T, T_H = NWIN * tpw_t, NWIN * tpw_h
    wimg = build_wimg(w)
    rel16 = build_rel16(w)
    bimg = build_bimg(w)
    exd = [build_example({n: per[n][b] for n in names}, w, tpw_t, tpw_h)
           for b in range(B)]
    in_maps = []
    for c in range(B // 2):
        im = build_core_inputs(exd[2 * c], exd[2 * c + 1], wimg, T_T, T_H)
        im["rel16"] = rel16
        im["bimg"] = bimg
        in_maps.append(im)
    e_mask = np.stack([e["e_mask"] for e in exd])
    return in_maps, T_T, T_H, rel16.shape, e_mask


_IN_NAMES = ["wimg", "mimg", "idx16", "leimg", "qemb2", "rel16", "bimg"]
_CACHE_DIR = os.environ.get(
    "BKERN_CACHE", os.path.expanduser("~/.cache/nnx58454504899045"))
_MEM = {}


def _export_path(key):
    return os.path.join(_CACHE_DIR, f"exp_{key}.bin")



def _install_cc_cache():
    """neuronx-cc hook + NEFF-bytes disk cache keyed on the HLO, so a fresh
    process skips the multi-second walrus compile."""
    from concourse.bass2jax import install_neuronx_cc_hook
    install_neuronx_cc_hook()
    import hashlib
    import libneuronxla
    if getattr(libneuronxla, "_bkern_cc_cached", False):
        return
    inner = libneuronxla.neuronx_cc

    def cached(code, code_format, platform_version, file_prefix):
        key = hashlib.sha256(b"bk1" + code).hexdigest()
        path = os.path.join(_CACHE_DIR, f"neff_{key}.bin")
        try:
            if os.path.exists(path):
                return 0, open(path, "rb").read()
        except Exception:
            pass
        r = inner(code, code_format, platform_version, file_prefix)
        try:
            if r[0] == 0 and isinstance(r[1], (bytes, bytearray)):
                os.makedirs(_CACHE_DIR, exist_ok=True)
                tmp = path + f".tmp{os.getpid()}"
                open(tmp, "wb").write(r[1])
                os.replace(tmp, path)
        except Exception:
            pass
        return r

    libneuronxla.neuronx_cc = cached
    libneuronxla._bkern_cc_cached = True


def _build_exported(T_T, T_H, rel16_shape, sample_maps):
    import jax
    from jax.sharding import Mesh, PartitionSpec
    from jax.experimental.shard_map import shard_map
    from concourse.bass2jax import (
        _bass_exec_p, partition_id_tensor, _fast_dispatch_active)
    _install_cc_cache()
    nc = build_nc(T_T, T_H, rel16_shape)
    pn = nc.partition_id_tensor.name if nc.partition_id_tensor else None
    out_name, out_shape = "out", (2, EP)
    out_avals = [jax.core.ShapedArray(out_shape, np.float32)]
    in_names_full = list(_IN_NAMES) + [out_name] + ([pn] if pn else [])

    def _body(*args):
        operands = list(args)
        if pn:
            operands.append(partition_id_tensor())
        outs = _bass_exec_p.bind(
            *operands, out_avals=tuple(out_avals),
            in_names=tuple(in_names_full), out_names=(out_name,),
            lowering_input_output_aliases=(), sim_require_finite=True,
            sim_require_nnan=True, nc=nc)
        return tuple(outs)

    devices = jax.devices()[:8]
    mesh = Mesh(np.asarray(devices), ("core",))
    nin = len(_IN_NAMES)
    f = jax.jit(
        shard_map(_body, mesh=mesh,
                  in_specs=(PartitionSpec("core"),) * (nin + 1),
                  out_specs=(PartitionSpec("core"),), check_rep=False),
        donate_argnums=(nin,), keep_unused=True)
    gl = [np.concatenate([m[n] for m in sample_maps], axis=0)
          for n in _IN_NAMES]
    zout = np.zeros((8 * 2, EP), np.float32)
    with _fast_dispatch_active(True):
        exported = jax.export.export(
            f, disabled_checks=[
                jax.export.DisabledSafetyCheck.custom_call("bass_exec")]
        )(*gl, zout)
    return exported


def _get_exported(T_T, T_H, rel16_shape, sample_maps):
    key = f"{T_T}_{T_H}_{rel16_shape[0]}_v7"
    if key in _MEM:
        return _MEM[key]
    import jax
    path = _export_path(key)
    exported = None
    if os.path.exists(path):
        try:
            _install_cc_cache()
            exported = jax.export.deserialize(open(path, "rb").read())
        except Exception:
            exported = None
    if exported is None:
        built = _build_exported(T_T, T_H, rel16_shape, sample_maps)
        blob = built.serialize()
        try:
            os.makedirs(_CACHE_DIR, exist_ok=True)
            tmp = path + ".tmp"
            open(tmp, "wb").write(blob)
            os.replace(tmp, path)
        except Exception:
            pass
        # Always hand out the DESERIALIZED artifact: its StableHLO bytes are
        # what future processes will compile, so the NEFF cache key matches
        # across processes (the freshly-built jaxpr lowers to slightly
        # different HLO and would fork the cache).
        exported = jax.export.deserialize(blob)
    _MEM[key] = exported
    return exported


def run_device(in_maps, T_T, T_H, rel16_shape):
    import jax
    from jax.sharding import Mesh, PartitionSpec, NamedSharding
    exported = _get_exported(T_T, T_H, rel16_shape, in_maps)
    mesh = Mesh(np.asarray(jax.devices()[:8]), ("core",))
    sh = NamedSharding(mesh, PartitionSpec("core"))
    gl = [jax.device_put(np.concatenate([m[n] for m in in_maps], axis=0), sh)
          for n in _IN_NAMES]
    zout = jax.device_put(np.zeros((8 * 2, EP), np.float32), sh)
    callf = jax.jit(exported.call)
    res = callf(*gl, zout)
    return np.asarray(res[0])


def kernel(**inputs):
    in_maps, T_T, T_H, rel16_shape, e_mask = prepare(inputs)
    out = run_device(in_maps, T_T, T_H, rel16_shape)   # (16, EP)
    score = out[:, :E]
    return (score + (1.0 - e_mask) * VERY_NEG).astype(np.float32)
